# revision 17
# baseline (speedup 1.0000x reference)
"""Trainium2 Bass kernel for nn_DeconvBlockTransformer.

Data-parallel over batch: B=16 sharded as 2 images per NeuronCore across 8
cores.  Weights/constants replicated.  One Bass program runs SPMD on all 8
cores via run_bass_kernel_spmd; per-core outputs are concatenated on axis 0.

Per-core pipeline (2 images):
  1. maxpool(2x2) on skip tensor b                        -> bpool  (SBUF)
  2. x2 = x + pe1_h (broadcast over w)                    -> SBUF (f32r)
  3. height-axis attention (per (b,w) column group)       -> b2 (SBUF), attn_h
  4. x3 = x2 + pe1_w (broadcast over h), spill x3 to DRAM
  5. width-axis attention (per (b,h) row group)           -> out (DRAM), attn_w
  6. g = sigmoid(bn(Wa@out + ba))                         -> SBUF
  7. fused row-pipeline per image:  bilinear-up(x3) -> conv3x3(Wup)+ReLU = xu;
     bilinear-up(g) * b = b_out;  conv3x3(Wc1) on [xu; b_out] +BN+ReLU -> y1;
     conv3x3(Wc2)+BN+ReLU -> y  (streamed in 8-row chunks, PSUM matmuls)

Matmuls use float32r (TF32-like, ~1.2e-4 relative rounding) where the moving
dim is >=256 (Q/K projections, all convolutions, gating); exact float32
elsewhere (logits, attention-value, transposes).
"""

import numpy as np

import concourse.bacc as bacc
import concourse.mybir as mybir
from concourse.tile import TileContext
from concourse.bass_utils import run_bass_kernel_spmd

F32 = mybir.dt.float32
F32R = mybir.dt.float32r
AT = mybir.ActivationFunctionType
ALU = mybir.AluOpType
AX = mybir.AxisListType

B, D1, H1, W1 = 16, 256, 64, 64
D2, H2, W2 = 128, 128, 128
EPS = 1e-5
NCORES = 8
BL = B // NCORES  # images per core

DEBUG_TAPS = False  # extra DRAM dumps for bring-up


# ----------------------------------------------------------------- host math
def _pe2d(d, H, W):
    dh = d // 2
    div = np.exp(np.arange(0, dh, 2) * -(np.log(10000.0) / dh))
    sw = np.sin(np.arange(W)[:, None] * div).T
    cw = np.cos(np.arange(W)[:, None] * div).T
    sh = np.sin(np.arange(H)[:, None] * div).T
    ch = np.cos(np.arange(H)[:, None] * div).T
    pe = np.zeros((d, H, W), np.float32)
    pe[0:dh:2] = np.broadcast_to(sw[:, None, :], (dh // 2, H, W))
    pe[1:dh:2] = np.broadcast_to(cw[:, None, :], (dh // 2, H, W))
    pe[dh::2] = np.broadcast_to(sh[:, :, None], (dh // 2, H, W))
    pe[dh + 1::2] = np.broadcast_to(ch[:, :, None], (dh // 2, H, W))
    return pe


def _upsample_tabs():
    # 2x bilinear, align_corners=True, 64 -> 128 (both axes).
    # even j=2k:   up[j] = src[k-1]*we0[k] + src[k]*we1[k]   (we0[0]=0)
    # odd  j=2k+1: up[j] = src[k]*wo0[k] + src[k+1]*wo1[k]   (wo1[63]=0)
    k = np.arange(64, dtype=np.float64)
    we0 = k / 127.0
    we1 = 1.0 - k / 127.0
    wo0 = (64.0 + k) / 127.0
    wo1 = (63.0 - k) / 127.0
    return np.stack([we0, we1, wo0, wo1]).astype(np.float32)  # [4, 64]


def _host_consts(w):
    """w: dict of full-model weights (np.float32). Returns replicated consts."""
    s = np.float32(1.0 / np.sqrt(1.0 + EPS))
    c = {}
    c["wqh"] = np.ascontiguousarray(w["Wq_h"].T).reshape(2, 128, 256)
    c["wkh"] = np.ascontiguousarray(w["Wk_h"].T).reshape(2, 128, 256)
    c["wqw"] = np.ascontiguousarray(w["Wq_w"].T).reshape(2, 128, 256)
    c["wkw"] = np.ascontiguousarray(w["Wk_w"].T).reshape(2, 128, 256)
    c["wvh"] = np.ascontiguousarray(w["Wv_h"].T)
    c["wvw"] = np.ascontiguousarray(w["Wv_w"].T)
    c["waT"] = np.ascontiguousarray(w["Wa"].T)
    pe1_h = _pe2d(D1, H1, 1)[:, :, 0]
    pe2_h = _pe2d(D2, H2 // 2, 1)[:, :, 0]
    pe1_w = _pe2d(D1, 1, H1)[:, 0, :]
    pe2_w = _pe2d(D2, 1, H2 // 2)[:, 0, :]
    c["pe1h"] = pe1_h.reshape(2, 128, 64)
    c["pe1w"] = pe1_w.reshape(2, 128, 64)
    c["pe2h"] = pe2_h  # [128, 64]
    c["pe2w"] = pe2_w
    c["wup"] = np.ascontiguousarray(w["Wup"].transpose(2, 3, 1, 0)).reshape(3, 3, 2, 128, 128)
    c["wc1"] = np.ascontiguousarray(w["Wc1"].transpose(2, 3, 1, 0)).reshape(3, 3, 2, 128, 128)
    c["wc2"] = np.ascontiguousarray(w["Wc2"].transpose(2, 3, 1, 0)).reshape(3, 3, 128, 128)
    c["v_bup"] = w["bup"].reshape(128, 1)
    gs = w["bn_a_g"] * s
    c["v_gs"] = gs.reshape(128, 1)
    c["v_gb"] = (w["ba"] * gs + w["bn_a_b"]).reshape(128, 1)
    c1s = w["bn1_g"] * s
    c["v_c1s"] = c1s.reshape(128, 1)
    c["v_c1b"] = (w["bc1"] * c1s + w["bn1_b"]).reshape(128, 1)
    c2s = w["bn2_g"] * s
    c["v_c2s"] = c2s.reshape(128, 1)
    c["v_c2b"] = (w["bc2"] * c2s + w["bn2_b"]).reshape(128, 1)
    c["wtab"] = np.broadcast_to(_upsample_tabs()[None], (128, 4, 64)).copy()
    c["ident"] = np.eye(128, dtype=np.float32)
    return {k_: np.ascontiguousarray(v_, dtype=np.float32) for k_, v_ in c.items()}


# H-interp schedule: for output row t (0..127): up[t] = s0*src[a] + s1*src[a+1]
# (a may be -1 with s0 == 0, or a+1 == 64 with s1 == 0; zero rows are padded).
def _hsched(t):
    if t % 2 == 0:
        m = t // 2
        return m - 1, float(m / 127.0), float(1.0 - m / 127.0)
    m = (t - 1) // 2
    return m, float((64.0 + m) / 127.0), float((63.0 - m) / 127.0)


# ------------------------------------------------------------- program build
def build_program():
    nc = bacc.Bacc("TRN2", target_bir_lowering=False)

    # ---- I/O ----
    x_dr = nc.dram_tensor("x", [BL, D1, H1, W1], F32R, kind="ExternalInput")
    b_dr = nc.dram_tensor("b", [BL, D2, H2, W2], F32, kind="ExternalInput")
    wqh_dr = nc.dram_tensor("wqh", [2, 128, 256], F32R, kind="ExternalInput")
    wkh_dr = nc.dram_tensor("wkh", [2, 128, 256], F32R, kind="ExternalInput")
    wqw_dr = nc.dram_tensor("wqw", [2, 128, 256], F32R, kind="ExternalInput")
    wkw_dr = nc.dram_tensor("wkw", [2, 128, 256], F32R, kind="ExternalInput")
    wvh_dr = nc.dram_tensor("wvh", [128, 128], F32, kind="ExternalInput")
    wvw_dr = nc.dram_tensor("wvw", [128, 128], F32, kind="ExternalInput")
    waT_dr = nc.dram_tensor("waT", [128, 128], F32R, kind="ExternalInput")
    pe1h_dr = nc.dram_tensor("pe1h", [2, 128, 64], F32, kind="ExternalInput")
    pe1w_dr = nc.dram_tensor("pe1w", [2, 128, 64], F32, kind="ExternalInput")
    pe2h_dr = nc.dram_tensor("pe2h", [128, 64], F32, kind="ExternalInput")
    pe2w_dr = nc.dram_tensor("pe2w", [128, 64], F32, kind="ExternalInput")
    wup_dr = nc.dram_tensor("wup", [3, 3, 2, 128, 128], F32R, kind="ExternalInput")
    wc1_dr = nc.dram_tensor("wc1", [3, 3, 2, 128, 128], F32R, kind="ExternalInput")
    wc2_dr = nc.dram_tensor("wc2", [3, 3, 128, 128], F32R, kind="ExternalInput")
    vecs_dr = {}
    for nm in ["v_bup", "v_gs", "v_gb", "v_c1s", "v_c1b", "v_c2s", "v_c2b"]:
        vecs_dr[nm] = nc.dram_tensor(nm, [128, 1], F32, kind="ExternalInput")
    wtab_dr = nc.dram_tensor("wtab", [128, 4, 64], F32, kind="ExternalInput")
    ident_dr = nc.dram_tensor("ident", [128, 128], F32, kind="ExternalInput")

    y_dr = nc.dram_tensor("y", [BL, D2, H2, W2], F32, kind="ExternalOutput")
    ah_dr = nc.dram_tensor("attn_h", [BL * W1, H1, H1], F32, kind="ExternalOutput")
    aw_dr = nc.dram_tensor("attn_w", [BL * H1, W1, W1], F32, kind="ExternalOutput")

    # internal DRAM scratch
    x3_dr = nc.dram_tensor("x3sc", [BL, 2, 128, 64, 66], F32R)  # [b, cc, c, h, w+pads]
    out_dr = nc.dram_tensor("outsc", [BL, 128, 64, 64], F32R)

    dbg = {}
    if DEBUG_TAPS:
        dbg["bpool"] = nc.dram_tensor("dbg_bpool", [BL, 128, 64, 64], F32, kind="ExternalOutput")
        dbg["b2"] = nc.dram_tensor("dbg_b2", [BL, 128, 64, 64], F32, kind="ExternalOutput")
        dbg["g"] = nc.dram_tensor("dbg_g", [BL, 128, 64, 64], F32, kind="ExternalOutput")
        dbg["xu"] = nc.dram_tensor("dbg_xu", [BL, 128, H2, W2], F32, kind="ExternalOutput")
        dbg["bout"] = nc.dram_tensor("dbg_bout", [BL, 128, H2, W2], F32, kind="ExternalOutput")

    with TileContext(nc) as tc:
        cpool = tc.alloc_tile_pool(name="cpool", bufs=1)
        # ---- load constants ----
        wqh_t = cpool.tile([128, 2, 256], F32R)
        wkh_t = cpool.tile([128, 2, 256], F32R)
        wqw_t = cpool.tile([128, 2, 256], F32R)
        wkw_t = cpool.tile([128, 2, 256], F32R)
        for t_, d_ in [(wqh_t, wqh_dr), (wkh_t, wkh_dr), (wqw_t, wqw_dr), (wkw_t, wkw_dr)]:
            nc.sync.dma_start(t_[:], d_.rearrange("c p o -> p c o"))
        wvh_t = cpool.tile([128, 128], F32)
        wvw_t = cpool.tile([128, 128], F32)
        waT_t = cpool.tile([128, 128], F32R)
        nc.sync.dma_start(wvh_t[:], wvh_dr[:])
        nc.sync.dma_start(wvw_t[:], wvw_dr[:])
        nc.sync.dma_start(waT_t[:], waT_dr[:])
        pe1h_t = cpool.tile([128, 2, 64], F32)
        pe1w_t = cpool.tile([128, 2, 64], F32)
        nc.sync.dma_start(pe1h_t[:], pe1h_dr.rearrange("c p o -> p c o"))
        nc.sync.dma_start(pe1w_t[:], pe1w_dr.rearrange("c p o -> p c o"))
        pe2h_t = cpool.tile([128, 64], F32)
        pe2w_t = cpool.tile([128, 64], F32)
        nc.sync.dma_start(pe2h_t[:], pe2h_dr[:])
        nc.sync.dma_start(pe2w_t[:], pe2w_dr[:])
        wup_t = cpool.tile([128, 3, 3, 2, 128], F32R)
        wc1_t = cpool.tile([128, 3, 3, 2, 128], F32R)
        wc2_t = cpool.tile([128, 3, 3, 128], F32R)
        nc.sync.dma_start(wup_t[:], wup_dr.rearrange("ky kx c p o -> p ky kx c o"))
        nc.sync.dma_start(wc1_t[:], wc1_dr.rearrange("ky kx c p o -> p ky kx c o"))
        nc.sync.dma_start(wc2_t[:], wc2_dr.rearrange("ky kx p o -> p ky kx o"))
        vec_t = {}
        for nm, d_ in vecs_dr.items():
            vec_t[nm] = cpool.tile([128, 1], F32, name=f"t{nm}")
            nc.sync.dma_start(vec_t[nm][:], d_[:])
        wtab_t = cpool.tile([128, 4, 64], F32)
        nc.sync.dma_start(wtab_t[:], wtab_dr[:])
        ident_t = cpool.tile([128, 128], F32)
        nc.sync.dma_start(ident_t[:], ident_dr[:])
        zrow_t = cpool.tile([128, 256], F32R)
        nc.gpsimd.memset(zrow_t.bitcast(F32)[:], 0.0)

        # zero the pad columns (0 and 65) of the x3 DRAM scratch once
        nc.sync.dma_start(x3_dr[:, :, :, :, 0].rearrange("b c p h -> (b c p h)")[None, :]
                          .rearrange("o (p f) -> (o p) f", p=128), zrow_t[:])
        nc.sync.dma_start(x3_dr[:, :, :, :, 65].rearrange("b c p h -> (b c p h)")[None, :]
                          .rearrange("o (p f) -> (o p) f", p=128), zrow_t[:])

        b2pool = tc.alloc_tile_pool(name="b2pool", bufs=1)
        b2_t = b2pool.tile([128, BL, 64, 64], F32)  # [c, b, h, w]

        bppool = tc.alloc_tile_pool(name="bppool", bufs=1)
        bpool_t = bppool.tile([128, BL, 64, 64], F32)  # [c, b, h, w]

        # ================= phase 1: maxpool on b =================
        with nc.named_scope("maxpool"), tc.tile_pool(name="mp", bufs=2) as mp:
            for bi in range(BL):
                for quad in range(4):  # 32 input rows each
                    bq = mp.tile([128, 32, 128], F32, tag="bq")
                    nc.sync.dma_start(bq[:], b_dr[bi, :, 32 * quad:32 * quad + 32, :])
                    th = mp.tile([128, 32, 64], F32, tag="th")
                    nc.vector.tensor_tensor(th[:], bq[:, :, 0:128:2], bq[:, :, 1:128:2], ALU.max)
                    nc.vector.tensor_tensor(
                        bpool_t[:, bi, 16 * quad:16 * quad + 16, :],
                        th[:, 0:32:2, :], th[:, 1:32:2, :], ALU.max)

        # fold the v-projection positional bias: bpool += pe2_h (bcast over w)
        for bi in range(BL):
            nc.vector.tensor_tensor(
                bpool_t[:, bi], bpool_t[:, bi],
                pe2h_t[:, :, None].to_broadcast([128, 64, 64]), ALU.add)
        if DEBUG_TAPS:
            for bi in range(BL):
                nc.sync.dma_start(dbg["bpool"][bi], bpool_t[:, bi])

        # ================= phase 2: x2 = x + pe1_h =================
        xpool = tc.alloc_tile_pool(name="xpool", bufs=1)
        x2 = []
        for bi in range(BL):
            x2_t = xpool.tile([128, 2, 64, 66], F32R, name=f"x2_{bi}")
            for cc in range(2):
                nc.gpsimd.memset(x2_t.bitcast(F32)[:, cc, :, 0], 0.0)
                nc.gpsimd.memset(x2_t.bitcast(F32)[:, cc, :, 65], 0.0)
            x2.append(x2_t)
            for cc in range(2):
                nc.sync.dma_start(x2_t[:, cc, :, 1:65],
                                  x_dr[bi, 128 * cc:128 * cc + 128])
            # += pe1_h (broadcast over w)
            nc.vector.tensor_tensor(
                x2_t[:, :, :, 1:65], x2_t.bitcast(F32)[:, :, :, 1:65],
                pe1h_t[:, :, :, None].to_broadcast([128, 2, 64, 64]), ALU.add)

        # ======== attention pass helper ========
        def attention_pass(axis):
            """axis='h': per-(b,w) column attention; axis='w': per-(b,h) row."""
            if axis == "h":
                wq_t, wk_t, wv_t, pe2_t, attn_out = wqh_t, wkh_t, wvh_t, pe2h_t, ah_dr
            else:
                wq_t, wk_t, wv_t, pe2_t, attn_out = wqw_t, wkw_t, wvw_t, pe2w_t, aw_dr

            with tc.tile_pool(name=f"ap_{axis}", bufs=2) as wp, \
                 tc.tile_pool(name=f"aps_{axis}", bufs=1, space="PSUM") as pp:
                for bi in range(BL):
                    for sg in range(8):  # supergroups of 8 groups
                        g0 = 8 * sg
                        # ---- Q, K projections: psum [o, 8 groups, 64 pos]
                        qk_sb = []
                        for wt_ in (wq_t, wk_t):
                            sb_ = wp.tile([128, 2, 8, 64], F32, tag="qk_sb",
                                          name=f"qk{axis}{bi}{sg}")
                            for oc in range(2):
                                ps_ = pp.tile([128, 8, 64], F32, tag="qk", bufs=2,
                                              name="ps_qk")
                                for cc in range(2):
                                    if axis == "h":
                                        rhs = x2[bi][:, cc, :, 1 + g0:1 + g0 + 8] \
                                            .transpose([0, 2, 1])
                                    else:
                                        rhs = x2[bi][:, cc, g0:g0 + 8, 1:65]
                                    nc.tensor.matmul(
                                        ps_[:], wt_[:, cc, 128 * oc:128 * oc + 128],
                                        rhs, start=(cc == 0), stop=(cc == 1))
                                nc.scalar.copy(sb_[:, oc], ps_[:])
                            qk_sb.append(sb_)
                        q_sb, k_sb = qk_sb

                        for pr in range(4):  # pairs of groups
                            ga = g0 + 2 * pr
                            # ---- vT = (src_col + pe2)^T @ WvT  per group
                            vps = pp.tile([64, 2, 128], F32, tag="vt", bufs=2,
                                          name="ps_vt")
                            for gi in range(2):
                                if axis == "h":
                                    lhs_main = bpool_t[:, bi, :, ga + gi]
                                else:
                                    lhs_main = b2_t[:, bi, ga + gi, :]
                                nc.tensor.matmul(vps[:, gi, :], lhs_main, wv_t[:],
                                                 start=True, stop=True)
                            vt_sb = wp.tile([64, 2, 128], F32, tag="vt_sb")
                            nc.vector.tensor_copy(vt_sb[:], vps[:])

                            # ---- logits, both groups packed on partitions
                            lps = pp.tile([128, 64], F32, tag="L", bufs=1, name="ps_L")
                            for gi in range(2):
                                for oc in range(2):
                                    nc.tensor.matmul(
                                        lps[64 * gi:64 * gi + 64, :],
                                        q_sb[:, oc, 2 * pr + gi, :].bitcast(F32),
                                        k_sb[:, oc, 2 * pr + gi, :].bitcast(F32),
                                        start=(oc == 0), stop=(oc == 1),
                                        tile_position=(0, 64 * gi))
                            # ---- softmax over free dim
                            negmx = wp.tile([128, 1], F32, tag="negmx")
                            nc.vector.tensor_reduce(negmx, lps[:], AX.X, ALU.max,
                                                    negate=True)
                            pex = wp.tile([128, 64], F32, tag="pex")
                            sums = wp.tile([128, 1], F32, tag="sums")
                            nc.scalar.activation(pex[:], lps[:], AT.Exp,
                                                 bias=negmx[:], scale=1.0,
                                                 accum_out=sums[:])
                            rec = wp.tile([128, 1], F32, tag="rec")
                            nc.vector.reciprocal(rec, sums[:])
                            attn = wp.tile([128, 64], F32, tag="attn")
                            nc.vector.tensor_scalar_mul(attn[:], pex[:], rec[:])
                            row = bi * 64 + ga
                            nc.sync.dma_start(
                                attn_out[row:row + 2].rearrange("g i j -> (g i) j"),
                                attn[:])
                            # ---- attn^T via PE transpose: [j, (g, i)]
                            atps = pp.tile([64, 128], F32, tag="at", bufs=1,
                                           name="ps_at")
                            nc.tensor.transpose(atps[:], attn[:], ident_t[:])
                            at_sb = wp.tile([64, 128], F32, tag="at_sb")
                            nc.vector.tensor_copy(at_sb[:], atps[:])
                            # ---- out = vT^T @ attnT  -> [c, i] per group
                            ops_ = pp.tile([128, 2, 64], F32, tag="ops", bufs=2,
                                           name="ps_o")
                            for gi in range(2):
                                nc.tensor.matmul(ops_[:, gi, :], vt_sb[:, gi, :],
                                                 at_sb[:, 64 * gi:64 * gi + 64],
                                                 start=True, stop=True)
                            if axis == "h":
                                # b2[c, bi, :, ga+gi] <- out (i == h)
                                nc.scalar.copy(
                                    b2_t[:, bi, :, ga:ga + 2].transpose([0, 2, 1]),
                                    ops_[:])
                            else:
                                ost = wp.tile([128, 2, 64], F32R, tag="ost")
                                nc.scalar.copy(ost[:], ops_[:])
                                nc.sync.dma_start(
                                    out_dr[bi, :, ga:ga + 2, :], ost[:])

        # ================= phase 3: height attention =================
        with nc.named_scope("attn_h"):
            attention_pass("h")
        if DEBUG_TAPS:
            for bi in range(BL):
                nc.sync.dma_start(dbg["b2"][bi], b2_t[:, bi])

        # fold width-pass v positional bias: b2 += pe2_w (bcast over h)
        for bi in range(BL):
            nc.vector.tensor_tensor(
                b2_t[:, bi], b2_t[:, bi],
                pe2w_t[:, None, :].to_broadcast([128, 64, 64]), ALU.add)

        # ================= phase 4: x3 = x2 + pe1_w; spill =================
        for bi in range(BL):
            nc.vector.tensor_tensor(
                x2[bi][:, :, :, 1:65], x2[bi].bitcast(F32)[:, :, :, 1:65],
                pe1w_t[:, :, None, :].to_broadcast([128, 2, 64, 64]), ALU.add)

        # ================= phase 5: width attention =================
        with nc.named_scope("attn_w"):
            attention_pass("w")

        # spill x3 to DRAM (data cols only; pads pre-zeroed)
        for bi in range(BL):
            for cc in range(2):
                nc.sync.dma_start(x3_dr[bi, cc, :, :, 1:65], x2[bi][:, cc, :, 1:65])

        xpool.release()
        bppool.release()
        b2pool.release()

        # ================= phase 6+7: gating + fused conv pipeline =========
        NCH = 16            # chunks per image
        RS = 8              # rows per chunk
        with tc.tile_pool(name="cv", bufs=2) as cv, \
             tc.tile_pool(name="gpool", bufs=1) as gp, \
             tc.tile_pool(name="cps", bufs=1, space="PSUM") as cps:
            for bi in range(BL):
                sc = nc.enter_named_scope(f"conv_{bi}", False)
                # ---- gating: g = sigmoid(bn(Wa@out + ba)) -> g_pad [c,66,66]
                g_pad = gp.tile([128, 66, 66], F32, tag="g_pad")
                nc.gpsimd.memset(g_pad[:], 0.0)
                for q in range(8):
                    go = cv.tile([128, 8, 64], F32R, tag="go")
                    nc.sync.dma_start(go[:], out_dr[bi, :, 8 * q:8 * q + 8, :])
                    gps = cps.tile([128, 8, 64], F32, tag="g", bufs=2, name="ps_g")
                    nc.tensor.matmul(gps[:], waT_t[:], go[:], start=True, stop=True)
                    nc.scalar.activation(g_pad[:, 1 + 8 * q:9 + 8 * q, 1:65], gps[:],
                                         AT.Sigmoid, bias=vec_t["v_gb"][:],
                                         scale=vec_t["v_gs"][:])

                if DEBUG_TAPS:
                    nc.sync.dma_start(dbg["g"][bi], g_pad[:, 1:65, 1:65])

                # chunk tiles kept by index for boundary-row copies
                upc_tiles, catc_tiles, y1c_tiles = {}, {}, {}

                def produce_up_chunk(j):
                    """A(j): up rows 8j..8j+7 -> upc slots 1..8 (+slot0 copy)."""
                    xc = cv.tile([128, 2, 6, 66], F32R, tag="xc", name="xc")
                    r_lo = 4 * j - 1
                    if j == 0:
                        for cc in range(2):
                            nc.gpsimd.memset(xc.bitcast(F32)[:, cc, 0, :], 0.0)
                        nc.sync.dma_start(xc[:, :, 1:6, :],
                                          x3_dr[bi, :, :, 0:5, :].rearrange("c p r w -> p c r w"))
                    elif j == 15:
                        for cc in range(2):
                            nc.gpsimd.memset(xc.bitcast(F32)[:, cc, 5, :], 0.0)
                        nc.sync.dma_start(xc[:, :, 0:5, :],
                                          x3_dr[bi, :, :, 59:64, :].rearrange("c p r w -> p c r w"))
                    else:
                        nc.sync.dma_start(xc[:],
                                          x3_dr[bi, :, :, r_lo:r_lo + 6, :].rearrange("c p r w -> p c r w"))
                    m0 = 4 * j
                    tall = cv.tile([128, 2, 8, 66], F32, tag="tall", name="tall")
                    # h-lerp evens (rows 2m: src rows m-1, m -> slots 0..3 / 1..4)
                    wa_ = wtab_t[:, 0, m0:m0 + 4][:, None, :, None] \
                        .to_broadcast([128, 2, 4, 66])
                    wb_ = wtab_t[:, 1, m0:m0 + 4][:, None, :, None] \
                        .to_broadcast([128, 2, 4, 66])
                    t1 = cv.tile([128, 2, 4, 66], F32, tag="hl1", name="hl1")
                    nc.vector.tensor_tensor(t1[:], xc.bitcast(F32)[:, :, 0:4, :], wa_, ALU.mult)
                    nc.vector.tensor_tensor(tall[:, :, 0:8:2, :],
                                            xc.bitcast(F32)[:, :, 1:5, :], wb_, ALU.mult)
                    nc.vector.tensor_tensor(tall[:, :, 0:8:2, :],
                                            tall[:, :, 0:8:2, :], t1[:], ALU.add)
                    # h-lerp odds (rows 2m+1: src rows m, m+1 -> slots 1..4 / 2..5)
                    wc_ = wtab_t[:, 2, m0:m0 + 4][:, None, :, None] \
                        .to_broadcast([128, 2, 4, 66])
                    wd_ = wtab_t[:, 3, m0:m0 + 4][:, None, :, None] \
                        .to_broadcast([128, 2, 4, 66])
                    t3 = cv.tile([128, 2, 4, 66], F32, tag="hl3", name="hl3")
                    nc.gpsimd.tensor_tensor(t3[:], xc.bitcast(F32)[:, :, 1:5, :], wc_, ALU.mult)
                    nc.gpsimd.tensor_tensor(tall[:, :, 1:8:2, :],
                                            xc.bitcast(F32)[:, :, 2:6, :], wd_, ALU.mult)
                    nc.gpsimd.tensor_tensor(tall[:, :, 1:8:2, :],
                                            tall[:, :, 1:8:2, :], t3[:], ALU.add)

                    upc = cv.tile([128, 2, 10, 130], F32R, tag="upc", name="upc")
                    # w-lerp evens -> cols 1,3,..,127; odds -> cols 2,4,..,128
                    we0 = wtab_t[:, 0, :][:, None, None, :].to_broadcast([128, 2, 8, 64])
                    we1 = wtab_t[:, 1, :][:, None, None, :].to_broadcast([128, 2, 8, 64])
                    wo0 = wtab_t[:, 2, :][:, None, None, :].to_broadcast([128, 2, 8, 64])
                    wo1 = wtab_t[:, 3, :][:, None, None, :].to_broadcast([128, 2, 8, 64])
                    u1 = cv.tile([128, 2, 8, 64], F32, tag="wl1", name="wl1")
                    ue = upc[:, :, 1:9, 1:129:2]
                    nc.vector.tensor_tensor(u1[:], tall[:, :, :, 0:64], we0, ALU.mult)
                    nc.vector.tensor_tensor(ue, tall[:, :, :, 1:65], we1, ALU.mult)
                    nc.vector.tensor_tensor(ue, ue.bitcast(F32), u1[:], ALU.add)
                    u3 = cv.tile([128, 2, 8, 64], F32, tag="wl3", name="wl3")
                    uo = upc[:, :, 1:9, 2:130:2]
                    nc.gpsimd.tensor_tensor(u3[:], tall[:, :, :, 1:65], wo0, ALU.mult)
                    nc.gpsimd.tensor_tensor(uo, tall[:, :, :, 2:66], wo1, ALU.mult)
                    nc.gpsimd.tensor_tensor(uo, uo.bitcast(F32), u3[:], ALU.add)
                    # pad cols 0 and 129 of fresh slot rows
                    for cc in range(2):
                        nc.gpsimd.memset(upc.bitcast(F32)[:, cc, :, 0], 0.0)
                        nc.gpsimd.memset(upc.bitcast(F32)[:, cc, :, 129], 0.0)
                    # slot 0 <- previous chunk's slot 8 (row 8j-1)
                    if j == 0:
                        for cc in range(2):
                            nc.gpsimd.memset(upc.bitcast(F32)[:, cc, 0, 1:129], 0.0)
                    else:
                        nc.gpsimd.tensor_copy(upc[:, :, 0:1, 1:129],
                                              upc_tiles[j - 1][:, :, 8:9, 1:129])
                    upc_tiles[j] = upc
                    return upc

                def conv_chunk(jj, src_tiles, w_t, nchunks_c, epi):
                    """3x3 conv over src window chunk jj; epi(psum, q) writes out.
                    src window tile: [128, (ncc,) 10, 130] slots 1..8 = rows
                    8jj..8jj+7; slot 9 boundary filled here."""
                    win = src_tiles[jj]
                    ncc = nchunks_c
                    # slot 9 <- next chunk slot 1 (row 8jj+8) or zeros
                    if jj == 15:
                        if ncc == 2:
                            for cc in range(2):
                                nc.gpsimd.memset(win.bitcast(F32)[:, cc, 9, 1:129], 0.0)
                        else:
                            nc.gpsimd.memset(win.bitcast(F32)[:, 9, 1:129], 0.0)
                    else:
                        nxt = src_tiles[jj + 1]
                        if ncc == 2:
                            nc.gpsimd.tensor_copy(win[:, :, 9:10, 1:129],
                                                  nxt[:, :, 1:2, 1:129])
                        else:
                            nc.gpsimd.tensor_copy(win[:, 9:10, 1:129],
                                                  nxt[:, 1:2, 1:129])
                    for q in range(2):  # two 4-row batches
                        ps = cps.tile([128, 4, 128], F32, tag=epi["ptag"], bufs=2,
                                      name=f"ps_{epi['ptag']}")
                        first = True
                        for dy in range(3):
                            for dx in range(3):
                                for cc in range(ncc):
                                    if ncc == 2:
                                        rhs = win[:, cc, 4 * q + dy:4 * q + dy + 4,
                                                  dx:dx + 128]
                                        lhs = w_t[:, dy, dx, cc, :]
                                    else:
                                        rhs = win[:, 4 * q + dy:4 * q + dy + 4,
                                                  dx:dx + 128]
                                        lhs = w_t[:, dy, dx, :]
                                    last = (dy == 2 and dx == 2 and cc == ncc - 1)
                                    nc.tensor.matmul(ps[:], lhs, rhs,
                                                     start=first, stop=last)
                                    first = False
                        epi["fn"](ps, q)

                def make_cat_chunk(jj):
                    """B(jj): conv-up + b_out for rows 8jj..8jj+7 -> catc."""
                    catc = cv.tile([128, 2, 10, 130], F32R, tag="catc", name="catc")
                    for cc in range(2):
                        nc.gpsimd.memset(catc.bitcast(F32)[:, cc, :, 0], 0.0)
                        nc.gpsimd.memset(catc.bitcast(F32)[:, cc, :, 129], 0.0)
                    if jj == 0:
                        for cc in range(2):
                            nc.gpsimd.memset(catc.bitcast(F32)[:, cc, 0, 1:129], 0.0)
                    else:
                        nc.gpsimd.tensor_copy(catc[:, :, 0:1, 1:129],
                                              catc_tiles[jj - 1][:, :, 8:9, 1:129])
                    catc_tiles[jj] = catc

                    def epi_up(ps, q):
                        nc.scalar.activation(
                            catc[:, 0, 1 + 4 * q:5 + 4 * q, 1:129], ps[:],
                            AT.Relu, bias=vec_t["v_bup"][:], scale=1.0)
                        if DEBUG_TAPS:
                            xu_sb = cv.tile([128, 4, 128], F32, tag="dbgxu")
                            nc.vector.tensor_copy(xu_sb[:], catc.bitcast(F32)[:, 0, 1 + 4 * q:5 + 4 * q, 1:129])
                            nc.sync.dma_start(
                                dbg["xu"][bi, :, 8 * jj + 4 * q:8 * jj + 4 * q + 4, :], xu_sb[:])
                    conv_chunk(jj, upc_tiles, wup_t, 2,
                               {"ptag": "up", "fn": epi_up})

                    # ---- b_out rows 8jj..8jj+7
                    m0 = 4 * jj
                    tg = cv.tile([128, 8, 66], F32, tag="tg", name="tg")
                    wa_ = wtab_t[:, 0, m0:m0 + 4][:, :, None].to_broadcast([128, 4, 66])
                    wb_ = wtab_t[:, 1, m0:m0 + 4][:, :, None].to_broadcast([128, 4, 66])
                    wc_ = wtab_t[:, 2, m0:m0 + 4][:, :, None].to_broadcast([128, 4, 66])
                    wd_ = wtab_t[:, 3, m0:m0 + 4][:, :, None].to_broadcast([128, 4, 66])
                    g1 = cv.tile([128, 4, 66], F32, tag="gl1", name="gl1")
                    nc.vector.tensor_tensor(g1[:], g_pad[:, m0:m0 + 4, :], wa_, ALU.mult)
                    nc.vector.tensor_tensor(tg[:, 0:8:2, :],
                                            g_pad[:, m0 + 1:m0 + 5, :], wb_, ALU.mult)
                    nc.vector.tensor_tensor(tg[:, 0:8:2, :], tg[:, 0:8:2, :], g1[:], ALU.add)
                    g3 = cv.tile([128, 4, 66], F32, tag="gl3", name="gl3")
                    nc.gpsimd.tensor_tensor(g3[:], g_pad[:, m0 + 1:m0 + 5, :], wc_, ALU.mult)
                    nc.gpsimd.tensor_tensor(tg[:, 1:8:2, :],
                                            g_pad[:, m0 + 2:m0 + 6, :], wd_, ALU.mult)
                    nc.gpsimd.tensor_tensor(tg[:, 1:8:2, :], tg[:, 1:8:2, :], g3[:], ALU.add)
                    upg = cv.tile([128, 8, 130], F32, tag="upg", name="upg")
                    we0 = wtab_t[:, 0, :][:, None, :].to_broadcast([128, 8, 64])
                    we1 = wtab_t[:, 1, :][:, None, :].to_broadcast([128, 8, 64])
                    wo0 = wtab_t[:, 2, :][:, None, :].to_broadcast([128, 8, 64])
                    wo1 = wtab_t[:, 3, :][:, None, :].to_broadcast([128, 8, 64])
                    q1 = cv.tile([128, 8, 64], F32, tag="ql1", name="ql1")
                    ge = upg[:, :, 1:129:2]
                    nc.vector.tensor_tensor(q1[:], tg[:, :, 0:64], we0, ALU.mult)
                    nc.vector.tensor_tensor(ge, tg[:, :, 1:65], we1, ALU.mult)
                    nc.vector.tensor_tensor(ge, ge, q1[:], ALU.add)
                    q3 = cv.tile([128, 8, 64], F32, tag="ql3", name="ql3")
                    go_ = upg[:, :, 2:130:2]
                    nc.gpsimd.tensor_tensor(q3[:], tg[:, :, 1:65], wo0, ALU.mult)
                    nc.gpsimd.tensor_tensor(go_, tg[:, :, 2:66], wo1, ALU.mult)
                    nc.gpsimd.tensor_tensor(go_, go_, q3[:], ALU.add)
                    bt = cv.tile([128, 8, 128], F32, tag="bt", name="bt")
                    nc.sync.dma_start(bt[:], b_dr[bi, :, 8 * jj:8 * jj + 8, :])
                    nc.vector.tensor_tensor(catc[:, 1, 1:9, 1:129],
                                            upg[:, :, 1:129], bt[:], ALU.mult)
                    if DEBUG_TAPS:
                        bo_sb = cv.tile([128, 8, 128], F32, tag="dbgbo")
                        nc.vector.tensor_copy(bo_sb[:], catc.bitcast(F32)[:, 1, 1:9, 1:129])
                        nc.sync.dma_start(dbg["bout"][bi, :, 8 * jj:8 * jj + 8, :], bo_sb[:])

                def make_y1_chunk(jj):
                    y1c = cv.tile([128, 10, 130], F32R, tag="y1c", name="y1c")
                    nc.gpsimd.memset(y1c.bitcast(F32)[:, :, 0], 0.0)
                    nc.gpsimd.memset(y1c.bitcast(F32)[:, :, 129], 0.0)
                    if jj == 0:
                        nc.gpsimd.memset(y1c.bitcast(F32)[:, 0:1, 1:129], 0.0)
                    else:
                        nc.gpsimd.tensor_copy(y1c[:, 0:1, 1:129],
                                              y1c_tiles[jj - 1][:, 8:9, 1:129])
                    y1c_tiles[jj] = y1c

                    def epi_c1(ps, q):
                        nc.scalar.activation(y1c[:, 1 + 4 * q:5 + 4 * q, 1:129],
                                             ps[:], AT.Relu,
                                             bias=vec_t["v_c1b"][:],
                                             scale=vec_t["v_c1s"][:])
                    conv_chunk(jj, catc_tiles, wc1_t, 2,
                               {"ptag": "c1", "fn": epi_c1})

                def make_y_chunk(jj):
                    yst = cv.tile([128, 8, 128], F32, tag="yst", name="yst")

                    def epi_c2(ps, q):
                        nc.scalar.activation(yst[:, 4 * q:4 * q + 4, :], ps[:],
                                             AT.Relu, bias=vec_t["v_c2b"][:],
                                             scale=vec_t["v_c2s"][:])
                    conv_chunk(jj, y1c_tiles, wc2_t, 1,
                               {"ptag": "c2", "fn": epi_c2})
                    nc.sync.dma_start(y_dr[bi, :, 8 * jj:8 * jj + 8, :], yst[:])

                # ---- 4-deep chunk-skewed pipeline
                for j in range(NCH + 3):
                    if j < NCH:
                        produce_up_chunk(j)
                    if 1 <= j <= NCH:
                        make_cat_chunk(j - 1)
                    if 2 <= j <= NCH + 1:
                        make_y1_chunk(j - 2)
                    if 3 <= j <= NCH + 2:
                        make_y_chunk(j - 3)
                nc.leave_named_scope(f"conv_{bi}", sc[0], False)
        cpool.release()
    nc.compile()
    return nc


# ------------------------------------------------------------------ runtime
_CACHE = {}


def _get_program():
    if "nc" not in _CACHE:
        _CACHE["nc"] = build_program()
    return _CACHE["nc"]


def kernel(**inputs):
    inputs = {k: np.asarray(v, dtype=np.float32) for k, v in inputs.items()}
    nc = _get_program()
    consts = _host_consts(inputs)
    in_maps = []
    for c in range(NCORES):
        m = dict(consts)
        m["x"] = inputs["x"][c * BL:(c + 1) * BL]
        m["b"] = inputs["b"][c * BL:(c + 1) * BL]
        in_maps.append(m)
    res = run_bass_kernel_spmd(nc, in_maps, core_ids=list(range(NCORES)),
                               trace=False)
    y = np.concatenate([res.results[c]["y"] for c in range(NCORES)], axis=0)
    ah = np.concatenate([res.results[c]["attn_h"] for c in range(NCORES)], axis=0)
    aw = np.concatenate([res.results[c]["attn_w"] for c in range(NCORES)], axis=0)
    _CACHE["last_results"] = res
    return y, ah, aw


# revision 25
# speedup vs baseline: 1.2527x; 1.2527x over previous
"""Trainium2 Bass kernel for nn_DeconvBlockTransformer.

Data-parallel over batch: B=16 sharded as 2 images per NeuronCore across 8
cores.  Weights/constants replicated.  One Bass program runs SPMD on all 8
cores via run_bass_kernel_spmd; per-core outputs are concatenated on axis 0.

Per-core pipeline (2 images):
  1. maxpool(2x2) on skip tensor b                        -> bpool  (SBUF)
  2. x2 = x + pe1_h (broadcast over w)                    -> SBUF (f32r)
  3. height-axis attention (per (b,w) column group)       -> b2 (SBUF), attn_h
  4. x3 = x2 + pe1_w (broadcast over h), spill x3 to DRAM
  5. width-axis attention (per (b,h) row group)           -> out (DRAM), attn_w
  6. g = sigmoid(bn(Wa@out + ba))                         -> SBUF
  7. fused row-pipeline per image:  bilinear-up(x3) -> conv3x3(Wup)+ReLU = xu;
     bilinear-up(g) * b = b_out;  conv3x3(Wc1) on [xu; b_out] +BN+ReLU -> y1;
     conv3x3(Wc2)+BN+ReLU -> y  (streamed in 8-row chunks, PSUM matmuls)

Matmuls use float32r (TF32-like, ~1.2e-4 relative rounding) where the moving
dim is >=256 (Q/K projections, all convolutions, gating); exact float32
elsewhere (logits, attention-value, transposes).
"""

import numpy as np

import concourse.bacc as bacc
import concourse.mybir as mybir
from concourse.tile import TileContext
from concourse.bass_utils import run_bass_kernel_spmd

F32 = mybir.dt.float32
F32R = mybir.dt.float32r
AT = mybir.ActivationFunctionType
ALU = mybir.AluOpType
AX = mybir.AxisListType

B, D1, H1, W1 = 16, 256, 64, 64
D2, H2, W2 = 128, 128, 128
EPS = 1e-5
NCORES = 8
BL = B // NCORES  # images per core

DEBUG_TAPS = False  # extra DRAM dumps for bring-up


# ----------------------------------------------------------------- host math
def _pe2d(d, H, W):
    dh = d // 2
    div = np.exp(np.arange(0, dh, 2) * -(np.log(10000.0) / dh))
    sw = np.sin(np.arange(W)[:, None] * div).T
    cw = np.cos(np.arange(W)[:, None] * div).T
    sh = np.sin(np.arange(H)[:, None] * div).T
    ch = np.cos(np.arange(H)[:, None] * div).T
    pe = np.zeros((d, H, W), np.float32)
    pe[0:dh:2] = np.broadcast_to(sw[:, None, :], (dh // 2, H, W))
    pe[1:dh:2] = np.broadcast_to(cw[:, None, :], (dh // 2, H, W))
    pe[dh::2] = np.broadcast_to(sh[:, :, None], (dh // 2, H, W))
    pe[dh + 1::2] = np.broadcast_to(ch[:, :, None], (dh // 2, H, W))
    return pe


def _upsample_tabs():
    # 2x bilinear, align_corners=True, 64 -> 128 (both axes).
    # even j=2k:   up[j] = src[k-1]*we0[k] + src[k]*we1[k]   (we0[0]=0)
    # odd  j=2k+1: up[j] = src[k]*wo0[k] + src[k+1]*wo1[k]   (wo1[63]=0)
    k = np.arange(64, dtype=np.float64)
    we0 = k / 127.0
    we1 = 1.0 - k / 127.0
    wo0 = (64.0 + k) / 127.0
    wo1 = (63.0 - k) / 127.0
    return np.stack([we0, we1, wo0, wo1]).astype(np.float32)  # [4, 64]


def _host_consts(w):
    """w: dict of full-model weights (np.float32). Returns replicated consts."""
    s = np.float32(1.0 / np.sqrt(1.0 + EPS))
    c = {}
    c["wqh"] = np.ascontiguousarray(w["Wq_h"].T).reshape(2, 128, 256)
    c["wkh"] = np.ascontiguousarray(w["Wk_h"].T).reshape(2, 128, 256)
    c["wqw"] = np.ascontiguousarray(w["Wq_w"].T).reshape(2, 128, 256)
    c["wkw"] = np.ascontiguousarray(w["Wk_w"].T).reshape(2, 128, 256)
    c["wvh"] = np.ascontiguousarray(w["Wv_h"].T)
    c["wvw"] = np.ascontiguousarray(w["Wv_w"].T)
    c["waT"] = np.ascontiguousarray(w["Wa"].T)
    pe1_h = _pe2d(D1, H1, 1)[:, :, 0]
    pe2_h = _pe2d(D2, H2 // 2, 1)[:, :, 0]
    pe1_w = _pe2d(D1, 1, H1)[:, 0, :]
    pe2_w = _pe2d(D2, 1, H2 // 2)[:, 0, :]
    c["pe1h"] = pe1_h.reshape(2, 128, 64)
    c["pe1w"] = pe1_w.reshape(2, 128, 64)
    c["pe2h"] = pe2_h  # [128, 64]
    c["pe2w"] = pe2_w
    c["wup"] = np.ascontiguousarray(w["Wup"].transpose(2, 3, 1, 0)).reshape(3, 3, 2, 128, 128)
    c["wc1"] = np.ascontiguousarray(w["Wc1"].transpose(2, 3, 1, 0)).reshape(3, 3, 2, 128, 128)
    c["wc2"] = np.ascontiguousarray(w["Wc2"].transpose(2, 3, 1, 0)).reshape(3, 3, 128, 128)
    c["v_bup"] = w["bup"].reshape(128, 1)
    gs = w["bn_a_g"] * s
    c["v_gs"] = gs.reshape(128, 1)
    c["v_gb"] = (w["ba"] * gs + w["bn_a_b"]).reshape(128, 1)
    c1s = w["bn1_g"] * s
    c["v_c1s"] = c1s.reshape(128, 1)
    c["v_c1b"] = (w["bc1"] * c1s + w["bn1_b"]).reshape(128, 1)
    c2s = w["bn2_g"] * s
    c["v_c2s"] = c2s.reshape(128, 1)
    c["v_c2b"] = (w["bc2"] * c2s + w["bn2_b"]).reshape(128, 1)
    c["wtab"] = np.broadcast_to(_upsample_tabs()[None], (128, 4, 64)).copy()
    c["ident"] = np.eye(128, dtype=np.float32)
    c["ident2"] = np.vstack([np.eye(64, dtype=np.float32)] * 2)
    return {k_: np.ascontiguousarray(v_, dtype=np.float32) for k_, v_ in c.items()}


# H-interp schedule: for output row t (0..127): up[t] = s0*src[a] + s1*src[a+1]
# (a may be -1 with s0 == 0, or a+1 == 64 with s1 == 0; zero rows are padded).
def _hsched(t):
    if t % 2 == 0:
        m = t // 2
        return m - 1, float(m / 127.0), float(1.0 - m / 127.0)
    m = (t - 1) // 2
    return m, float((64.0 + m) / 127.0), float((63.0 - m) / 127.0)


# ------------------------------------------------------------- program build
def build_program():
    nc = bacc.Bacc("TRN2", target_bir_lowering=False)

    # ---- I/O ----
    x_dr = nc.dram_tensor("x", [BL, D1, H1, W1], F32R, kind="ExternalInput")
    b_dr = nc.dram_tensor("b", [BL, D2, H2, W2], F32, kind="ExternalInput")
    wqh_dr = nc.dram_tensor("wqh", [2, 128, 256], F32R, kind="ExternalInput")
    wkh_dr = nc.dram_tensor("wkh", [2, 128, 256], F32R, kind="ExternalInput")
    wqw_dr = nc.dram_tensor("wqw", [2, 128, 256], F32R, kind="ExternalInput")
    wkw_dr = nc.dram_tensor("wkw", [2, 128, 256], F32R, kind="ExternalInput")
    wvh_dr = nc.dram_tensor("wvh", [128, 128], F32, kind="ExternalInput")
    wvw_dr = nc.dram_tensor("wvw", [128, 128], F32, kind="ExternalInput")
    waT_dr = nc.dram_tensor("waT", [128, 128], F32R, kind="ExternalInput")
    pe1h_dr = nc.dram_tensor("pe1h", [2, 128, 64], F32, kind="ExternalInput")
    pe1w_dr = nc.dram_tensor("pe1w", [2, 128, 64], F32, kind="ExternalInput")
    pe2h_dr = nc.dram_tensor("pe2h", [128, 64], F32, kind="ExternalInput")
    pe2w_dr = nc.dram_tensor("pe2w", [128, 64], F32, kind="ExternalInput")
    wup_dr = nc.dram_tensor("wup", [3, 3, 2, 128, 128], F32R, kind="ExternalInput")
    wc1_dr = nc.dram_tensor("wc1", [3, 3, 2, 128, 128], F32R, kind="ExternalInput")
    wc2_dr = nc.dram_tensor("wc2", [3, 3, 128, 128], F32R, kind="ExternalInput")
    vecs_dr = {}
    for nm in ["v_bup", "v_gs", "v_gb", "v_c1s", "v_c1b", "v_c2s", "v_c2b"]:
        vecs_dr[nm] = nc.dram_tensor(nm, [128, 1], F32, kind="ExternalInput")
    wtab_dr = nc.dram_tensor("wtab", [128, 4, 64], F32, kind="ExternalInput")
    ident_dr = nc.dram_tensor("ident", [128, 128], F32, kind="ExternalInput")
    ident2_dr = nc.dram_tensor("ident2", [128, 64], F32, kind="ExternalInput")

    y_dr = nc.dram_tensor("y", [BL, D2, H2, W2], F32, kind="ExternalOutput")
    ah_dr = nc.dram_tensor("attn_h", [BL * W1, H1, H1], F32, kind="ExternalOutput")
    aw_dr = nc.dram_tensor("attn_w", [BL * H1, W1, W1], F32, kind="ExternalOutput")

    # internal DRAM scratch
    x3_dr = nc.dram_tensor("x3sc", [BL, 2, 128, 64, 66], F32R)  # [b, cc, c, h, w+pads]
    out_dr = nc.dram_tensor("outsc", [BL, 128, 64, 64], F32R)

    dbg = {}
    if DEBUG_TAPS:
        dbg["bpool"] = nc.dram_tensor("dbg_bpool", [BL, 128, 64, 64], F32, kind="ExternalOutput")
        dbg["b2"] = nc.dram_tensor("dbg_b2", [BL, 128, 64, 64], F32, kind="ExternalOutput")
        dbg["g"] = nc.dram_tensor("dbg_g", [BL, 128, 64, 64], F32, kind="ExternalOutput")
        dbg["xu"] = nc.dram_tensor("dbg_xu", [BL, 128, H2, W2], F32, kind="ExternalOutput")
        dbg["bout"] = nc.dram_tensor("dbg_bout", [BL, 128, H2, W2], F32, kind="ExternalOutput")

    with TileContext(nc) as tc:
        cpool = tc.alloc_tile_pool(name="cpool", bufs=1)
        # ---- load constants ----
        wqh_t = cpool.tile([128, 2, 256], F32R)
        wkh_t = cpool.tile([128, 2, 256], F32R)
        wqw_t = cpool.tile([128, 2, 256], F32R)
        wkw_t = cpool.tile([128, 2, 256], F32R)
        for t_, d_ in [(wqh_t, wqh_dr), (wkh_t, wkh_dr), (wqw_t, wqw_dr), (wkw_t, wkw_dr)]:
            nc.sync.dma_start(t_[:], d_.rearrange("c p o -> p c o"))
        wvh_t = cpool.tile([128, 128], F32)
        wvw_t = cpool.tile([128, 128], F32)
        waT_t = cpool.tile([128, 128], F32R)
        nc.sync.dma_start(wvh_t[:], wvh_dr[:])
        nc.sync.dma_start(wvw_t[:], wvw_dr[:])
        nc.sync.dma_start(waT_t[:], waT_dr[:])
        pe1h_t = cpool.tile([128, 2, 64], F32)
        pe1w_t = cpool.tile([128, 2, 64], F32)
        nc.sync.dma_start(pe1h_t[:], pe1h_dr.rearrange("c p o -> p c o"))
        nc.sync.dma_start(pe1w_t[:], pe1w_dr.rearrange("c p o -> p c o"))
        pe2h_t = cpool.tile([128, 64], F32)
        pe2w_t = cpool.tile([128, 64], F32)
        nc.sync.dma_start(pe2h_t[:], pe2h_dr[:])
        nc.sync.dma_start(pe2w_t[:], pe2w_dr[:])
        wup_t = cpool.tile([128, 3, 3, 2, 128], F32R)
        wc1_t = cpool.tile([128, 3, 3, 2, 128], F32R)
        wc2_t = cpool.tile([128, 3, 3, 128], F32R)
        nc.sync.dma_start(wup_t[:], wup_dr.rearrange("ky kx c p o -> p ky kx c o"))
        nc.sync.dma_start(wc1_t[:], wc1_dr.rearrange("ky kx c p o -> p ky kx c o"))
        nc.sync.dma_start(wc2_t[:], wc2_dr.rearrange("ky kx p o -> p ky kx o"))
        vec_t = {}
        for nm, d_ in vecs_dr.items():
            vec_t[nm] = cpool.tile([128, 1], F32, name=f"t{nm}")
            nc.sync.dma_start(vec_t[nm][:], d_[:])
        wtab_t = cpool.tile([128, 4, 64], F32)
        nc.sync.dma_start(wtab_t[:], wtab_dr[:])
        ident_t = cpool.tile([128, 128], F32)
        nc.sync.dma_start(ident_t[:], ident_dr[:])
        ident2_t = cpool.tile([128, 64], F32)
        nc.sync.dma_start(ident2_t[:], ident2_dr[:])

        b2pool = tc.alloc_tile_pool(name="b2pool", bufs=1)
        b2_t = b2pool.tile([128, BL, 64, 64], F32)  # [c, b, h, w]

        bppool = tc.alloc_tile_pool(name="bppool", bufs=1)
        bpool_t = bppool.tile([128, BL, 64, 64], F32)  # [c, b, h, w]

        # ================= phase 1: maxpool on b =================
        with nc.named_scope("maxpool"), tc.tile_pool(name="mp", bufs=2) as mp:
            for bi in range(BL):
                for quad in range(4):  # 32 input rows each
                    bq = mp.tile([128, 32, 128], F32, tag="bq")
                    nc.sync.dma_start(bq[:], b_dr[bi, :, 32 * quad:32 * quad + 32, :])
                    th = mp.tile([128, 32, 64], F32, tag="th")
                    nc.vector.tensor_tensor(th[:], bq[:, :, 0:128:2], bq[:, :, 1:128:2], ALU.max)
                    nc.vector.tensor_tensor(
                        bpool_t[:, bi, 16 * quad:16 * quad + 16, :],
                        th[:, 0:32:2, :], th[:, 1:32:2, :], ALU.max)

        # fold the v-projection positional bias: bpool += pe2_h (bcast over w)
        for bi in range(BL):
            nc.vector.tensor_tensor(
                bpool_t[:, bi], bpool_t[:, bi],
                pe2h_t[:, :, None].to_broadcast([128, 64, 64]), ALU.add)
        if DEBUG_TAPS:
            for bi in range(BL):
                nc.sync.dma_start(dbg["bpool"][bi], bpool_t[:, bi])

        # ================= phase 2: x2 = x + pe1_h =================
        xpool = tc.alloc_tile_pool(name="xpool", bufs=1)
        x2 = []
        for bi in range(BL):
            x2_t = xpool.tile([128, 2, 64, 66], F32R, name=f"x2_{bi}")
            for cc in range(2):
                nc.gpsimd.memset(x2_t.bitcast(F32)[:, cc, :, 0], 0.0)
                nc.gpsimd.memset(x2_t.bitcast(F32)[:, cc, :, 65], 0.0)
            x2.append(x2_t)
            for cc in range(2):
                nc.sync.dma_start(x2_t[:, cc, :, 1:65],
                                  x_dr[bi, 128 * cc:128 * cc + 128])
            # += pe1_h (broadcast over w)
            nc.vector.tensor_tensor(
                x2_t[:, :, :, 1:65], x2_t.bitcast(F32)[:, :, :, 1:65],
                pe1h_t[:, :, :, None].to_broadcast([128, 2, 64, 64]), ALU.add)

        # ======== attention pass helper ========
        def attention_pass(axis):
            """axis='h': per-(b,w) column attention; axis='w': per-(b,h) row.
            Supergroups of 16 groups; pairs packed on PSUM partition halves via
            tile_position col/row groups."""
            if axis == "h":
                wq_t, wk_t, wv_t, attn_out = wqh_t, wkh_t, wvh_t, ah_dr
            else:
                wq_t, wk_t, wv_t, attn_out = wqw_t, wkw_t, wvw_t, aw_dr

            with tc.tile_pool(name=f"ap_{axis}", bufs=2) as wp, \
                 tc.tile_pool(name=f"aps_{axis}", bufs=1, space="PSUM") as pp:
                for bi in range(BL):
                    for sg in range(4):  # supergroups of 16 groups
                        g0 = 16 * sg
                        # ---- Q, K: [o(2x128), 16 groups, 64 pos]
                        qk_sb = []
                        for ti, wt_ in enumerate((wq_t, wk_t)):
                            sb_ = wp.tile([128, 2, 16, 64], F32, tag="qk_sb",
                                          name=f"qk{axis}{bi}{sg}{ti}")
                            for oc in range(2):
                                for hf in range(2):
                                    ps_ = pp.tile([128, 8, 64], F32, tag="qk",
                                                  bufs=2, name="ps_qk")
                                    gh = g0 + 8 * hf
                                    for cc in range(2):
                                        if axis == "h":
                                            rhs = x2[bi][:, cc, :, 1 + gh:9 + gh] \
                                                .transpose([0, 2, 1])
                                        else:
                                            rhs = x2[bi][:, cc, gh:gh + 8, 1:65]
                                        nc.tensor.matmul(
                                            ps_[:], wt_[:, cc, 128 * oc:128 * oc + 128],
                                            rhs, start=(cc == 0), stop=(cc == 1))
                                    if ti == 0:
                                        nc.vector.tensor_copy(
                                            sb_[:, oc, 8 * hf:8 * hf + 8, :], ps_[:])
                                    else:
                                        nc.scalar.copy(
                                            sb_[:, oc, 8 * hf:8 * hf + 8, :], ps_[:])
                            qk_sb.append(sb_)
                        q_sb, k_sb = qk_sb

                        # ---- vT for all 16 groups (M=64 matmuls at partition 0)
                        vt_sb = wp.tile([64, 16, 128], F32, tag="vt_sb", bufs=1)
                        for half in range(2):
                            vps = pp.tile([64, 8, 128], F32, tag="vt", bufs=1,
                                          name="ps_vt")
                            for sl in range(8):
                                gg = g0 + 8 * half + sl
                                if axis == "h":
                                    lhs_main = bpool_t[:, bi, :, gg]
                                else:
                                    lhs_main = b2_t[:, bi, gg, :]
                                nc.tensor.matmul(vps[:, sl, :], lhs_main,
                                                 wv_t[:], start=True, stop=True)
                            nc.vector.tensor_copy(vt_sb[:, 8 * half:8 * half + 8, :],
                                                  vps[:])

                        # ---- logits for 8 pairs, i packed on partition halves
                        lps = pp.tile([128, 8, 64], F32, tag="L", bufs=1, name="ps_L")
                        for pr in range(8):
                            for gi in range(2):
                                for oc in range(2):
                                    nc.tensor.matmul(
                                        lps[64 * gi:64 * gi + 64, pr, :],
                                        q_sb[:, oc, 2 * pr + gi, :].bitcast(F32),
                                        k_sb[:, oc, 2 * pr + gi, :].bitcast(F32),
                                        start=(oc == 0), stop=(oc == 1),
                                        tile_position=(0, 64 * gi))
                        # ---- batched softmax over free dim
                        negmx = wp.tile([128, 8], F32, tag="negmx")
                        nc.vector.tensor_reduce(negmx, lps[:], AX.X, ALU.max,
                                                negate=True)
                        pex = wp.tile([128, 8, 64], F32, tag="pex")
                        nc.vector.tensor_tensor(
                            pex[:], lps[:],
                            negmx[:, :, None].to_broadcast([128, 8, 64]), ALU.add)
                        nc.scalar.activation(pex[:], pex[:], AT.Exp, bias=0.0,
                                             scale=1.0)
                        sums = wp.tile([128, 8], F32, tag="sums")
                        nc.vector.tensor_reduce(sums, pex[:], AX.X, ALU.add)
                        rec = wp.tile([128, 8], F32, tag="rec")
                        nc.vector.reciprocal(rec, sums[:])
                        attn = pex
                        nc.vector.tensor_tensor(
                            attn[:], pex[:],
                            rec[:, :, None].to_broadcast([128, 8, 64]), ALU.mult)
                        row = bi * 64 + g0
                        nc.sync.dma_start(
                            attn_out[row:row + 16]
                            .rearrange("(pr g) i j -> (g i) pr j", g=2),
                            attn[:])
                        # ---- attn^T per pair: full [128,64] -> [64,128] PE
                        atps = pp.tile([64, 8, 128], F32, tag="misc", bufs=1,
                                       name="ps_at")
                        for pr in range(8):
                            nc.tensor.transpose(atps[:, pr, :], attn[:, pr, :],
                                                ident_t[:])
                        at_sb = wp.tile([64, 8, 128], F32, tag="at_sb", bufs=1)
                        nc.scalar.copy(at_sb[:], atps[:])
                        # ---- out = vT^T @ attnT (K=64 at partition 0)
                        ops_ = pp.tile([128, 16, 64], F32, tag="misc", bufs=1,
                                       name="ps_o")
                        for pr in range(8):
                            for gi in range(2):
                                nc.tensor.matmul(ops_[:, 2 * pr + gi, :],
                                                 vt_sb[:, 2 * pr + gi, :],
                                                 at_sb[:, pr, 64 * gi:64 * gi + 64],
                                                 start=True, stop=True)
                        if axis == "h":
                            nc.scalar.copy(
                                b2_t[:, bi, :, g0:g0 + 16].transpose([0, 2, 1]),
                                ops_[:])
                        else:
                            ost = wp.tile([128, 16, 64], F32R, tag="ost", bufs=1)
                            nc.scalar.copy(ost[:], ops_[:])
                            nc.sync.dma_start(out_dr[bi, :, g0:g0 + 16, :], ost[:])

        # ================= phase 3: height attention =================
        with nc.named_scope("attn_h"):
            attention_pass("h")
        if DEBUG_TAPS:
            for bi in range(BL):
                nc.sync.dma_start(dbg["b2"][bi], b2_t[:, bi])

        # fold width-pass v positional bias: b2 += pe2_w (bcast over h)
        for bi in range(BL):
            nc.vector.tensor_tensor(
                b2_t[:, bi], b2_t[:, bi],
                pe2w_t[:, None, :].to_broadcast([128, 64, 64]), ALU.add)

        # ================= phase 4: x3 = x2 + pe1_w; spill =================
        for bi in range(BL):
            nc.vector.tensor_tensor(
                x2[bi][:, :, :, 1:65], x2[bi].bitcast(F32)[:, :, :, 1:65],
                pe1w_t[:, :, None, :].to_broadcast([128, 2, 64, 64]), ALU.add)

        # ================= phase 5: width attention =================
        with nc.named_scope("attn_w"):
            attention_pass("w")

        # spill x3 to DRAM (whole rows; pad cols are zero in SBUF)
        for bi in range(BL):
            for cc in range(2):
                nc.sync.dma_start(x3_dr[bi, cc], x2[bi][:, cc])

        xpool.release()
        bppool.release()
        b2pool.release()

        # ================= phase 6+7: gating + fused conv pipeline =========
        NCH = 16            # chunks per image
        RS = 8              # rows per chunk
        with tc.tile_pool(name="cv", bufs=3) as cv, \
             tc.tile_pool(name="gpool", bufs=1) as gp, \
             tc.tile_pool(name="cps", bufs=1, space="PSUM") as cps:
            for bi in range(BL):
                sc = nc.enter_named_scope(f"conv_{bi}", False)
                # ---- gating: g = sigmoid(bn(Wa@out + ba)) -> g_pad [c,66,66]
                g_pad = gp.tile([128, 66, 66], F32, tag="g_pad")
                nc.gpsimd.memset(g_pad[:], 0.0)
                for q in range(8):
                    go = cv.tile([128, 8, 64], F32R, tag="go", bufs=2)
                    nc.sync.dma_start(go[:], out_dr[bi, :, 8 * q:8 * q + 8, :])
                    gps = cps.tile([128, 8, 64], F32, tag="g", bufs=2, name="ps_g")
                    nc.tensor.matmul(gps[:], waT_t[:], go[:], start=True, stop=True)
                    nc.scalar.activation(g_pad[:, 1 + 8 * q:9 + 8 * q, 1:65], gps[:],
                                         AT.Sigmoid, bias=vec_t["v_gb"][:],
                                         scale=vec_t["v_gs"][:])

                if DEBUG_TAPS:
                    nc.sync.dma_start(dbg["g"][bi], g_pad[:, 1:65, 1:65])

                # chunk tiles kept by index for boundary-row copies
                upc_tiles, catc_tiles, y1c_tiles = {}, {}, {}

                def produce_up_chunk(j):
                    """A(j): up rows 8j..8j+7 -> upc slots 1..8 (+slot0 copy)."""
                    xc = cv.tile([128, 2, 6, 66], F32R, tag="xc", name="xc", bufs=2)
                    r_lo = 4 * j - 1
                    for cc in range(2):
                        if j == 0:
                            nc.gpsimd.memset(xc.bitcast(F32)[:, cc, 0, :], 0.0)
                            nc.sync.dma_start(xc[:, cc, 1:6, :],
                                              x3_dr[bi, cc, :, 0:5, :])
                        elif j == 15:
                            nc.gpsimd.memset(xc.bitcast(F32)[:, cc, 5, :], 0.0)
                            nc.sync.dma_start(xc[:, cc, 0:5, :],
                                              x3_dr[bi, cc, :, 59:64, :])
                        else:
                            nc.sync.dma_start(xc[:, cc],
                                              x3_dr[bi, cc, :, r_lo:r_lo + 6, :])
                    m0 = 4 * j
                    tall = cv.tile([128, 2, 8, 66], F32, tag="tall", name="tall", bufs=2)
                    # h-lerp evens (rows 2m: src rows m-1, m -> slots 0..3 / 1..4)
                    wa_ = wtab_t[:, 0, m0:m0 + 4][:, None, :, None] \
                        .to_broadcast([128, 2, 4, 66])
                    wb_ = wtab_t[:, 1, m0:m0 + 4][:, None, :, None] \
                        .to_broadcast([128, 2, 4, 66])
                    t1 = cv.tile([128, 2, 4, 66], F32, tag="hl1", name="hl1", bufs=2)
                    nc.vector.tensor_tensor(t1[:], xc.bitcast(F32)[:, :, 0:4, :], wa_, ALU.mult)
                    nc.vector.tensor_tensor(tall[:, :, 0:8:2, :],
                                            xc.bitcast(F32)[:, :, 1:5, :], wb_, ALU.mult)
                    nc.vector.tensor_tensor(tall[:, :, 0:8:2, :],
                                            tall[:, :, 0:8:2, :], t1[:], ALU.add)
                    # h-lerp odds (rows 2m+1: src rows m, m+1 -> slots 1..4 / 2..5)
                    wc_ = wtab_t[:, 2, m0:m0 + 4][:, None, :, None] \
                        .to_broadcast([128, 2, 4, 66])
                    wd_ = wtab_t[:, 3, m0:m0 + 4][:, None, :, None] \
                        .to_broadcast([128, 2, 4, 66])
                    t3 = cv.tile([128, 2, 4, 66], F32, tag="hl3", name="hl3", bufs=2)
                    nc.gpsimd.tensor_tensor(t3[:], xc.bitcast(F32)[:, :, 1:5, :], wc_, ALU.mult)
                    nc.gpsimd.tensor_tensor(tall[:, :, 1:8:2, :],
                                            xc.bitcast(F32)[:, :, 2:6, :], wd_, ALU.mult)
                    nc.gpsimd.tensor_tensor(tall[:, :, 1:8:2, :],
                                            tall[:, :, 1:8:2, :], t3[:], ALU.add)

                    upc = cv.tile([128, 2, 10, 130], F32R, tag="upc", name="upc")
                    # w-lerp evens -> cols 1,3,..,127; odds -> cols 2,4,..,128
                    we0 = wtab_t[:, 0, :][:, None, None, :].to_broadcast([128, 2, 8, 64])
                    we1 = wtab_t[:, 1, :][:, None, None, :].to_broadcast([128, 2, 8, 64])
                    wo0 = wtab_t[:, 2, :][:, None, None, :].to_broadcast([128, 2, 8, 64])
                    wo1 = wtab_t[:, 3, :][:, None, None, :].to_broadcast([128, 2, 8, 64])
                    u1 = cv.tile([128, 2, 8, 64], F32, tag="wl1", name="wl1", bufs=2)
                    ue = upc[:, :, 1:9, 1:129:2]
                    nc.vector.tensor_tensor(u1[:], tall[:, :, :, 0:64], we0, ALU.mult)
                    nc.vector.tensor_tensor(ue, tall[:, :, :, 1:65], we1, ALU.mult)
                    nc.vector.tensor_tensor(ue, ue.bitcast(F32), u1[:], ALU.add)
                    u3 = cv.tile([128, 2, 8, 64], F32, tag="wl3", name="wl3", bufs=2)
                    uo = upc[:, :, 1:9, 2:130:2]
                    nc.gpsimd.tensor_tensor(u3[:], tall[:, :, :, 1:65], wo0, ALU.mult)
                    nc.gpsimd.tensor_tensor(uo, tall[:, :, :, 2:66], wo1, ALU.mult)
                    nc.gpsimd.tensor_tensor(uo, uo.bitcast(F32), u3[:], ALU.add)
                    # pad cols 0 and 129 of fresh slot rows
                    for cc in range(2):
                        nc.gpsimd.memset(upc.bitcast(F32)[:, cc, :, 0], 0.0)
                        nc.gpsimd.memset(upc.bitcast(F32)[:, cc, :, 129], 0.0)
                    # slot 0 <- previous chunk's slot 8 (row 8j-1)
                    if j == 0:
                        for cc in range(2):
                            nc.gpsimd.memset(upc.bitcast(F32)[:, cc, 0, 1:129], 0.0)
                    else:
                        nc.gpsimd.tensor_copy(upc[:, :, 0:1, 1:129],
                                              upc_tiles[j - 1][:, :, 8:9, 1:129])
                    upc_tiles[j] = upc
                    return upc

                def conv_chunk(jj, src_tiles, w_t, nchunks_c, epi):
                    """3x3 conv over src window chunk jj; epi(psum, q) writes out.
                    src window tile: [128, (ncc,) 10, 130] slots 1..8 = rows
                    8jj..8jj+7; slot 9 boundary filled here."""
                    win = src_tiles[jj]
                    ncc = nchunks_c
                    # slot 9 <- next chunk slot 1 (row 8jj+8) or zeros
                    if jj == 15:
                        if ncc == 2:
                            for cc in range(2):
                                nc.gpsimd.memset(win.bitcast(F32)[:, cc, 9, 1:129], 0.0)
                        else:
                            nc.gpsimd.memset(win.bitcast(F32)[:, 9, 1:129], 0.0)
                    else:
                        nxt = src_tiles[jj + 1]
                        if ncc == 2:
                            nc.gpsimd.tensor_copy(win[:, :, 9:10, 1:129],
                                                  nxt[:, :, 1:2, 1:129])
                        else:
                            nc.gpsimd.tensor_copy(win[:, 9:10, 1:129],
                                                  nxt[:, 1:2, 1:129])
                    for q in range(2):  # two 4-row batches
                        ps = cps.tile([128, 4, 128], F32, tag=epi["ptag"], bufs=2,
                                      name=f"ps_{epi['ptag']}")
                        first = True
                        for dy in range(3):
                            for dx in range(3):
                                for cc in range(ncc):
                                    if ncc == 2:
                                        rhs = win[:, cc, 4 * q + dy:4 * q + dy + 4,
                                                  dx:dx + 128]
                                        lhs = w_t[:, dy, dx, cc, :]
                                    else:
                                        rhs = win[:, 4 * q + dy:4 * q + dy + 4,
                                                  dx:dx + 128]
                                        lhs = w_t[:, dy, dx, :]
                                    last = (dy == 2 and dx == 2 and cc == ncc - 1)
                                    nc.tensor.matmul(ps[:], lhs, rhs,
                                                     start=first, stop=last)
                                    first = False
                        epi["fn"](ps, q)

                def make_cat_chunk(jj):
                    """B(jj): conv-up + b_out for rows 8jj..8jj+7 -> catc."""
                    catc = cv.tile([128, 2, 10, 130], F32R, tag="catc", name="catc")
                    for cc in range(2):
                        nc.gpsimd.memset(catc.bitcast(F32)[:, cc, :, 0], 0.0)
                        nc.gpsimd.memset(catc.bitcast(F32)[:, cc, :, 129], 0.0)
                    if jj == 0:
                        for cc in range(2):
                            nc.gpsimd.memset(catc.bitcast(F32)[:, cc, 0, 1:129], 0.0)
                    else:
                        nc.gpsimd.tensor_copy(catc[:, :, 0:1, 1:129],
                                              catc_tiles[jj - 1][:, :, 8:9, 1:129])
                    catc_tiles[jj] = catc

                    def epi_up(ps, q):
                        nc.scalar.activation(
                            catc[:, 0, 1 + 4 * q:5 + 4 * q, 1:129], ps[:],
                            AT.Relu, bias=vec_t["v_bup"][:], scale=1.0)
                        if DEBUG_TAPS:
                            xu_sb = cv.tile([128, 4, 128], F32, tag="dbgxu")
                            nc.vector.tensor_copy(xu_sb[:], catc.bitcast(F32)[:, 0, 1 + 4 * q:5 + 4 * q, 1:129])
                            nc.sync.dma_start(
                                dbg["xu"][bi, :, 8 * jj + 4 * q:8 * jj + 4 * q + 4, :], xu_sb[:])
                    conv_chunk(jj, upc_tiles, wup_t, 2,
                               {"ptag": "up", "fn": epi_up})

                    # ---- b_out rows 8jj..8jj+7
                    m0 = 4 * jj
                    tg = cv.tile([128, 8, 66], F32, tag="tg", name="tg", bufs=2)
                    wa_ = wtab_t[:, 0, m0:m0 + 4][:, :, None].to_broadcast([128, 4, 66])
                    wb_ = wtab_t[:, 1, m0:m0 + 4][:, :, None].to_broadcast([128, 4, 66])
                    wc_ = wtab_t[:, 2, m0:m0 + 4][:, :, None].to_broadcast([128, 4, 66])
                    wd_ = wtab_t[:, 3, m0:m0 + 4][:, :, None].to_broadcast([128, 4, 66])
                    g1 = cv.tile([128, 4, 66], F32, tag="gl1", name="gl1", bufs=2)
                    nc.vector.tensor_tensor(g1[:], g_pad[:, m0:m0 + 4, :], wa_, ALU.mult)
                    nc.vector.tensor_tensor(tg[:, 0:8:2, :],
                                            g_pad[:, m0 + 1:m0 + 5, :], wb_, ALU.mult)
                    nc.vector.tensor_tensor(tg[:, 0:8:2, :], tg[:, 0:8:2, :], g1[:], ALU.add)
                    g3 = cv.tile([128, 4, 66], F32, tag="gl3", name="gl3", bufs=2)
                    nc.gpsimd.tensor_tensor(g3[:], g_pad[:, m0 + 1:m0 + 5, :], wc_, ALU.mult)
                    nc.gpsimd.tensor_tensor(tg[:, 1:8:2, :],
                                            g_pad[:, m0 + 2:m0 + 6, :], wd_, ALU.mult)
                    nc.gpsimd.tensor_tensor(tg[:, 1:8:2, :], tg[:, 1:8:2, :], g3[:], ALU.add)
                    upg = cv.tile([128, 8, 130], F32, tag="upg", name="upg", bufs=2)
                    we0 = wtab_t[:, 0, :][:, None, :].to_broadcast([128, 8, 64])
                    we1 = wtab_t[:, 1, :][:, None, :].to_broadcast([128, 8, 64])
                    wo0 = wtab_t[:, 2, :][:, None, :].to_broadcast([128, 8, 64])
                    wo1 = wtab_t[:, 3, :][:, None, :].to_broadcast([128, 8, 64])
                    q1 = cv.tile([128, 8, 64], F32, tag="ql1", name="ql1", bufs=2)
                    ge = upg[:, :, 1:129:2]
                    nc.vector.tensor_tensor(q1[:], tg[:, :, 0:64], we0, ALU.mult)
                    nc.vector.tensor_tensor(ge, tg[:, :, 1:65], we1, ALU.mult)
                    nc.vector.tensor_tensor(ge, ge, q1[:], ALU.add)
                    q3 = cv.tile([128, 8, 64], F32, tag="ql3", name="ql3", bufs=2)
                    go_ = upg[:, :, 2:130:2]
                    nc.gpsimd.tensor_tensor(q3[:], tg[:, :, 1:65], wo0, ALU.mult)
                    nc.gpsimd.tensor_tensor(go_, tg[:, :, 2:66], wo1, ALU.mult)
                    nc.gpsimd.tensor_tensor(go_, go_, q3[:], ALU.add)
                    bt = cv.tile([128, 8, 128], F32, tag="bt", name="bt", bufs=2)
                    nc.sync.dma_start(bt[:], b_dr[bi, :, 8 * jj:8 * jj + 8, :])
                    nc.vector.tensor_tensor(catc[:, 1, 1:9, 1:129],
                                            upg[:, :, 1:129], bt[:], ALU.mult)
                    if DEBUG_TAPS:
                        bo_sb = cv.tile([128, 8, 128], F32, tag="dbgbo")
                        nc.vector.tensor_copy(bo_sb[:], catc.bitcast(F32)[:, 1, 1:9, 1:129])
                        nc.sync.dma_start(dbg["bout"][bi, :, 8 * jj:8 * jj + 8, :], bo_sb[:])

                def make_y1_chunk(jj):
                    y1c = cv.tile([128, 10, 130], F32R, tag="y1c", name="y1c", bufs=2)
                    nc.gpsimd.memset(y1c.bitcast(F32)[:, :, 0], 0.0)
                    nc.gpsimd.memset(y1c.bitcast(F32)[:, :, 129], 0.0)
                    if jj == 0:
                        nc.gpsimd.memset(y1c.bitcast(F32)[:, 0:1, 1:129], 0.0)
                    else:
                        nc.gpsimd.tensor_copy(y1c[:, 0:1, 1:129],
                                              y1c_tiles[jj - 1][:, 8:9, 1:129])
                    y1c_tiles[jj] = y1c

                    def epi_c1(ps, q):
                        nc.scalar.activation(y1c[:, 1 + 4 * q:5 + 4 * q, 1:129],
                                             ps[:], AT.Relu,
                                             bias=vec_t["v_c1b"][:],
                                             scale=vec_t["v_c1s"][:])
                    conv_chunk(jj, catc_tiles, wc1_t, 2,
                               {"ptag": "c1", "fn": epi_c1})

                def make_y_chunk(jj):
                    yst = cv.tile([128, 8, 128], F32, tag="yst", name="yst", bufs=2)

                    def epi_c2(ps, q):
                        nc.scalar.activation(yst[:, 4 * q:4 * q + 4, :], ps[:],
                                             AT.Relu, bias=vec_t["v_c2b"][:],
                                             scale=vec_t["v_c2s"][:])
                    conv_chunk(jj, y1c_tiles, wc2_t, 1,
                               {"ptag": "c2", "fn": epi_c2})
                    nc.sync.dma_start(y_dr[bi, :, 8 * jj:8 * jj + 8, :], yst[:])

                # ---- 4-deep chunk-skewed pipeline
                for j in range(NCH + 3):
                    if j < NCH:
                        produce_up_chunk(j)
                    if 1 <= j <= NCH:
                        make_cat_chunk(j - 1)
                    if 2 <= j <= NCH + 1:
                        make_y1_chunk(j - 2)
                    if 3 <= j <= NCH + 2:
                        make_y_chunk(j - 3)
                nc.leave_named_scope(f"conv_{bi}", sc[0], False)
        cpool.release()
    nc.compile()
    return nc


# ------------------------------------------------------------------ runtime
_CACHE = {}


def _get_program():
    if "nc" not in _CACHE:
        _CACHE["nc"] = build_program()
    return _CACHE["nc"]


def kernel(**inputs):
    inputs = {k: np.asarray(v, dtype=np.float32) for k, v in inputs.items()}
    nc = _get_program()
    consts = _host_consts(inputs)
    in_maps = []
    for c in range(NCORES):
        m = dict(consts)
        m["x"] = inputs["x"][c * BL:(c + 1) * BL]
        m["b"] = inputs["b"][c * BL:(c + 1) * BL]
        in_maps.append(m)
    res = run_bass_kernel_spmd(nc, in_maps, core_ids=list(range(NCORES)),
                               trace=False)
    y = np.concatenate([res.results[c]["y"] for c in range(NCORES)], axis=0)
    ah = np.concatenate([res.results[c]["attn_h"] for c in range(NCORES)], axis=0)
    aw = np.concatenate([res.results[c]["attn_w"] for c in range(NCORES)], axis=0)
    _CACHE["last_results"] = res
    return y, ah, aw


# revision 28
# speedup vs baseline: 1.4528x; 1.1597x over previous
"""Trainium2 Bass kernel for nn_DeconvBlockTransformer.

Data-parallel over batch: B=16 sharded as 2 images per NeuronCore across 8
cores.  Weights/constants replicated.  One Bass program runs SPMD on all 8
cores via run_bass_kernel_spmd; per-core outputs are concatenated on axis 0.

Per-core pipeline (2 images):
  1. maxpool(2x2) on skip tensor b                        -> bpool  (SBUF)
  2. x2 = x + pe1_h (broadcast over w)                    -> SBUF (f32r)
  3. height-axis attention (per (b,w) column group)       -> b2 (SBUF), attn_h
  4. x3 = x2 + pe1_w (broadcast over h), spill x3 to DRAM
  5. width-axis attention (per (b,h) row group)           -> out (DRAM), attn_w
  6. g = sigmoid(bn(Wa@out + ba))                         -> SBUF
  7. fused row-pipeline per image:  bilinear-up(x3) -> conv3x3(Wup)+ReLU = xu;
     bilinear-up(g) * b = b_out;  conv3x3(Wc1) on [xu; b_out] +BN+ReLU -> y1;
     conv3x3(Wc2)+BN+ReLU -> y  (streamed in 8-row chunks, PSUM matmuls)

Matmuls use float32r (TF32-like, ~1.2e-4 relative rounding) where the moving
dim is >=256 (Q/K projections, all convolutions, gating); exact float32
elsewhere (logits, attention-value, transposes).
"""

import numpy as np

import concourse.bacc as bacc
import concourse.mybir as mybir
from concourse.tile import TileContext
from concourse.bass_utils import run_bass_kernel_spmd

F32 = mybir.dt.float32
F32R = mybir.dt.float32r
AT = mybir.ActivationFunctionType
ALU = mybir.AluOpType
AX = mybir.AxisListType

B, D1, H1, W1 = 16, 256, 64, 64
D2, H2, W2 = 128, 128, 128
EPS = 1e-5
NCORES = 8
BL = B // NCORES  # images per core

DEBUG_TAPS = False  # extra DRAM dumps for bring-up


# ----------------------------------------------------------------- host math
def _pe2d(d, H, W):
    dh = d // 2
    div = np.exp(np.arange(0, dh, 2) * -(np.log(10000.0) / dh))
    sw = np.sin(np.arange(W)[:, None] * div).T
    cw = np.cos(np.arange(W)[:, None] * div).T
    sh = np.sin(np.arange(H)[:, None] * div).T
    ch = np.cos(np.arange(H)[:, None] * div).T
    pe = np.zeros((d, H, W), np.float32)
    pe[0:dh:2] = np.broadcast_to(sw[:, None, :], (dh // 2, H, W))
    pe[1:dh:2] = np.broadcast_to(cw[:, None, :], (dh // 2, H, W))
    pe[dh::2] = np.broadcast_to(sh[:, :, None], (dh // 2, H, W))
    pe[dh + 1::2] = np.broadcast_to(ch[:, :, None], (dh // 2, H, W))
    return pe


def _upsample_tabs():
    # 2x bilinear, align_corners=True, 64 -> 128 (both axes).
    # even j=2k:   up[j] = src[k-1]*we0[k] + src[k]*we1[k]   (we0[0]=0)
    # odd  j=2k+1: up[j] = src[k]*wo0[k] + src[k+1]*wo1[k]   (wo1[63]=0)
    k = np.arange(64, dtype=np.float64)
    we0 = k / 127.0
    we1 = 1.0 - k / 127.0
    wo0 = (64.0 + k) / 127.0
    wo1 = (63.0 - k) / 127.0
    return np.stack([we0, we1, wo0, wo1]).astype(np.float32)  # [4, 64]


def _host_consts(w):
    """w: dict of full-model weights (np.float32). Returns replicated consts."""
    s = np.float32(1.0 / np.sqrt(1.0 + EPS))
    c = {}
    c["wqh"] = np.ascontiguousarray(w["Wq_h"].T).reshape(2, 128, 256)
    c["wkh"] = np.ascontiguousarray(w["Wk_h"].T).reshape(2, 128, 256)
    c["wqw"] = np.ascontiguousarray(w["Wq_w"].T).reshape(2, 128, 256)
    c["wkw"] = np.ascontiguousarray(w["Wk_w"].T).reshape(2, 128, 256)
    c["wvh"] = np.ascontiguousarray(w["Wv_h"].T)
    c["wvw"] = np.ascontiguousarray(w["Wv_w"].T)
    c["waT"] = np.ascontiguousarray(w["Wa"].T)
    pe1_h = _pe2d(D1, H1, 1)[:, :, 0]
    pe2_h = _pe2d(D2, H2 // 2, 1)[:, :, 0]
    pe1_w = _pe2d(D1, 1, H1)[:, 0, :]
    pe2_w = _pe2d(D2, 1, H2 // 2)[:, 0, :]
    c["pe1h"] = pe1_h.reshape(2, 128, 64)
    c["pe1w"] = pe1_w.reshape(2, 128, 64)
    c["pe2h"] = pe2_h  # [128, 64]
    c["pe2w"] = pe2_w
    c["wup"] = np.ascontiguousarray(w["Wup"].transpose(2, 3, 1, 0)).reshape(3, 3, 2, 128, 128)
    c["wc1"] = np.ascontiguousarray(w["Wc1"].transpose(2, 3, 1, 0)).reshape(3, 3, 2, 128, 128)
    c["wc2"] = np.ascontiguousarray(w["Wc2"].transpose(2, 3, 1, 0)).reshape(3, 3, 128, 128)
    c["v_bup"] = w["bup"].reshape(128, 1)
    gs = w["bn_a_g"] * s
    c["v_gs"] = gs.reshape(128, 1)
    c["v_gb"] = (w["ba"] * gs + w["bn_a_b"]).reshape(128, 1)
    c1s = w["bn1_g"] * s
    c["v_c1s"] = c1s.reshape(128, 1)
    c["v_c1b"] = (w["bc1"] * c1s + w["bn1_b"]).reshape(128, 1)
    c2s = w["bn2_g"] * s
    c["v_c2s"] = c2s.reshape(128, 1)
    c["v_c2b"] = (w["bc2"] * c2s + w["bn2_b"]).reshape(128, 1)
    c["wtab"] = np.broadcast_to(_upsample_tabs()[None], (128, 4, 64)).copy()
    c["ident"] = np.eye(128, dtype=np.float32)
    c["ident2"] = np.vstack([np.eye(64, dtype=np.float32)] * 2)
    return {k_: np.ascontiguousarray(v_, dtype=np.float32) for k_, v_ in c.items()}


# H-interp schedule: for output row t (0..127): up[t] = s0*src[a] + s1*src[a+1]
# (a may be -1 with s0 == 0, or a+1 == 64 with s1 == 0; zero rows are padded).
def _hsched(t):
    if t % 2 == 0:
        m = t // 2
        return m - 1, float(m / 127.0), float(1.0 - m / 127.0)
    m = (t - 1) // 2
    return m, float((64.0 + m) / 127.0), float((63.0 - m) / 127.0)


# ------------------------------------------------------------- program build
def build_program():
    nc = bacc.Bacc("TRN2", target_bir_lowering=False)

    # ---- I/O ----
    x_dr = nc.dram_tensor("x", [BL, D1, H1, W1], F32R, kind="ExternalInput")
    b_dr = nc.dram_tensor("b", [BL, D2, H2, W2], F32, kind="ExternalInput")
    wqh_dr = nc.dram_tensor("wqh", [2, 128, 256], F32R, kind="ExternalInput")
    wkh_dr = nc.dram_tensor("wkh", [2, 128, 256], F32R, kind="ExternalInput")
    wqw_dr = nc.dram_tensor("wqw", [2, 128, 256], F32R, kind="ExternalInput")
    wkw_dr = nc.dram_tensor("wkw", [2, 128, 256], F32R, kind="ExternalInput")
    wvh_dr = nc.dram_tensor("wvh", [128, 128], F32, kind="ExternalInput")
    wvw_dr = nc.dram_tensor("wvw", [128, 128], F32, kind="ExternalInput")
    waT_dr = nc.dram_tensor("waT", [128, 128], F32R, kind="ExternalInput")
    pe1h_dr = nc.dram_tensor("pe1h", [2, 128, 64], F32, kind="ExternalInput")
    pe1w_dr = nc.dram_tensor("pe1w", [2, 128, 64], F32, kind="ExternalInput")
    pe2h_dr = nc.dram_tensor("pe2h", [128, 64], F32, kind="ExternalInput")
    pe2w_dr = nc.dram_tensor("pe2w", [128, 64], F32, kind="ExternalInput")
    wup_dr = nc.dram_tensor("wup", [3, 3, 2, 128, 128], F32R, kind="ExternalInput")
    wc1_dr = nc.dram_tensor("wc1", [3, 3, 2, 128, 128], F32R, kind="ExternalInput")
    wc2_dr = nc.dram_tensor("wc2", [3, 3, 128, 128], F32R, kind="ExternalInput")
    vecs_dr = {}
    for nm in ["v_bup", "v_gs", "v_gb", "v_c1s", "v_c1b", "v_c2s", "v_c2b"]:
        vecs_dr[nm] = nc.dram_tensor(nm, [128, 1], F32, kind="ExternalInput")
    wtab_dr = nc.dram_tensor("wtab", [128, 4, 64], F32, kind="ExternalInput")
    ident_dr = nc.dram_tensor("ident", [128, 128], F32, kind="ExternalInput")
    ident2_dr = nc.dram_tensor("ident2", [128, 64], F32, kind="ExternalInput")

    y_dr = nc.dram_tensor("y", [BL, D2, H2, W2], F32, kind="ExternalOutput")
    ah_dr = nc.dram_tensor("attn_h", [BL * W1, H1, H1], F32, kind="ExternalOutput")
    aw_dr = nc.dram_tensor("attn_w", [BL * H1, W1, W1], F32, kind="ExternalOutput")

    # internal DRAM scratch
    x3_dr = nc.dram_tensor("x3sc", [BL, 2, 128, 64, 66], F32R)  # [b, cc, c, h, w+pads]
    out_dr = nc.dram_tensor("outsc", [BL, 128, 64, 64], F32R)

    dbg = {}
    if DEBUG_TAPS:
        dbg["bpool"] = nc.dram_tensor("dbg_bpool", [BL, 128, 64, 64], F32, kind="ExternalOutput")
        dbg["b2"] = nc.dram_tensor("dbg_b2", [BL, 128, 64, 64], F32, kind="ExternalOutput")
        dbg["g"] = nc.dram_tensor("dbg_g", [BL, 128, 64, 64], F32, kind="ExternalOutput")
        dbg["xu"] = nc.dram_tensor("dbg_xu", [BL, 128, H2, W2], F32, kind="ExternalOutput")
        dbg["bout"] = nc.dram_tensor("dbg_bout", [BL, 128, H2, W2], F32, kind="ExternalOutput")

    with TileContext(nc) as tc:
        cpool = tc.alloc_tile_pool(name="cpool", bufs=1)
        # ---- load constants ----
        wqh_t = cpool.tile([128, 2, 256], F32R)
        wkh_t = cpool.tile([128, 2, 256], F32R)
        wqw_t = cpool.tile([128, 2, 256], F32R)
        wkw_t = cpool.tile([128, 2, 256], F32R)
        for t_, d_ in [(wqh_t, wqh_dr), (wkh_t, wkh_dr), (wqw_t, wqw_dr), (wkw_t, wkw_dr)]:
            nc.sync.dma_start(t_[:], d_.rearrange("c p o -> p c o"))
        wvh_t = cpool.tile([128, 128], F32)
        wvw_t = cpool.tile([128, 128], F32)
        waT_t = cpool.tile([128, 128], F32R)
        nc.sync.dma_start(wvh_t[:], wvh_dr[:])
        nc.sync.dma_start(wvw_t[:], wvw_dr[:])
        nc.sync.dma_start(waT_t[:], waT_dr[:])
        pe1h_t = cpool.tile([128, 2, 64], F32)
        pe1w_t = cpool.tile([128, 2, 64], F32)
        nc.sync.dma_start(pe1h_t[:], pe1h_dr.rearrange("c p o -> p c o"))
        nc.sync.dma_start(pe1w_t[:], pe1w_dr.rearrange("c p o -> p c o"))
        pe2h_t = cpool.tile([128, 64], F32)
        pe2w_t = cpool.tile([128, 64], F32)
        nc.sync.dma_start(pe2h_t[:], pe2h_dr[:])
        nc.sync.dma_start(pe2w_t[:], pe2w_dr[:])
        wup_t = cpool.tile([128, 3, 3, 2, 128], F32R)
        wc1_t = cpool.tile([128, 3, 3, 2, 128], F32R)
        wc2_t = cpool.tile([128, 3, 3, 128], F32R)
        nc.sync.dma_start(wup_t[:], wup_dr.rearrange("ky kx c p o -> p ky kx c o"))
        nc.sync.dma_start(wc1_t[:], wc1_dr.rearrange("ky kx c p o -> p ky kx c o"))
        nc.sync.dma_start(wc2_t[:], wc2_dr.rearrange("ky kx p o -> p ky kx o"))
        vec_t = {}
        for nm, d_ in vecs_dr.items():
            vec_t[nm] = cpool.tile([128, 1], F32, name=f"t{nm}")
            nc.sync.dma_start(vec_t[nm][:], d_[:])
        wtab_t = cpool.tile([128, 4, 64], F32)
        nc.sync.dma_start(wtab_t[:], wtab_dr[:])
        ident_t = cpool.tile([128, 128], F32)
        nc.sync.dma_start(ident_t[:], ident_dr[:])
        ident2_t = cpool.tile([128, 64], F32)
        nc.sync.dma_start(ident2_t[:], ident2_dr[:])

        b2pool = tc.alloc_tile_pool(name="b2pool", bufs=1)
        b2_t = b2pool.tile([128, BL, 64, 64], F32)  # [c, b, h, w]

        bppool = tc.alloc_tile_pool(name="bppool", bufs=1)
        bpool_t = bppool.tile([128, BL, 64, 64], F32)  # [c, b, h, w]

        # ================= phase 2: x2 = x + pe1_h =================
        xpool = tc.alloc_tile_pool(name="xpool", bufs=1)
        x2 = []
        for bi in range(BL):
            x2_t = xpool.tile([128, 2, 64, 66], F32R, name=f"x2_{bi}")
            for cc in range(2):
                nc.gpsimd.memset(x2_t.bitcast(F32)[:, cc, :, 0], 0.0)
                nc.gpsimd.memset(x2_t.bitcast(F32)[:, cc, :, 65], 0.0)
            x2.append(x2_t)
            for cc in range(2):
                nc.sync.dma_start(x2_t[:, cc, :, 1:65],
                                  x_dr[bi, 128 * cc:128 * cc + 128])
            # += pe1_h (broadcast over w)
            nc.vector.tensor_tensor(
                x2_t[:, :, :, 1:65], x2_t.bitcast(F32)[:, :, :, 1:65],
                pe1h_t[:, :, :, None].to_broadcast([128, 2, 64, 64]), ALU.add)

        # ================= phase 1: maxpool on b =================
        with nc.named_scope("maxpool"), tc.tile_pool(name="mp", bufs=2) as mp:
            for bi in range(BL):
                for quad in range(8):  # 16 input rows each
                    bq = mp.tile([128, 16, 128], F32, tag="bq")
                    nc.sync.dma_start(bq[:], b_dr[bi, :, 16 * quad:16 * quad + 16, :])
                    th = mp.tile([128, 16, 64], F32, tag="th")
                    nc.vector.tensor_tensor(th[:], bq[:, :, 0:128:2], bq[:, :, 1:128:2], ALU.max)
                    nc.vector.tensor_tensor(
                        bpool_t[:, bi, 8 * quad:8 * quad + 8, :],
                        th[:, 0:16:2, :], th[:, 1:16:2, :], ALU.max)

        # fold the v-projection positional bias: bpool += pe2_h (bcast over w)
        for bi in range(BL):
            nc.vector.tensor_tensor(
                bpool_t[:, bi], bpool_t[:, bi],
                pe2h_t[:, :, None].to_broadcast([128, 64, 64]), ALU.add)
        if DEBUG_TAPS:
            for bi in range(BL):
                nc.sync.dma_start(dbg["bpool"][bi], bpool_t[:, bi])

        # ======== attention pass helper ========
        def attention_pass(axis):
            """axis='h': per-(b,w) column attention; axis='w': per-(b,h) row.
            Supergroups of 16 groups; pairs packed on PSUM partition halves via
            tile_position col/row groups."""
            if axis == "h":
                wq_t, wk_t, wv_t, attn_out = wqh_t, wkh_t, wvh_t, ah_dr
            else:
                wq_t, wk_t, wv_t, attn_out = wqw_t, wkw_t, wvw_t, aw_dr

            with tc.tile_pool(name=f"ap_{axis}", bufs=2) as wp, \
                 tc.tile_pool(name=f"aps_{axis}", bufs=1, space="PSUM") as pp:
                for bi in range(BL):
                    for sg in range(4):  # supergroups of 16 groups
                        g0 = 16 * sg
                        # ---- Q, K: [o(2x128), 16 groups, 64 pos]
                        qk_sb = []
                        for ti, wt_ in enumerate((wq_t, wk_t)):
                            sb_ = wp.tile([128, 2, 16, 64], F32, tag="qk_sb",
                                          name=f"qk{axis}{bi}{sg}{ti}")
                            for oc in range(2):
                                for hf in range(2):
                                    ps_ = pp.tile([128, 8, 64], F32, tag="qk",
                                                  bufs=2, name="ps_qk")
                                    gh = g0 + 8 * hf
                                    for cc in range(2):
                                        if axis == "h":
                                            rhs = x2[bi][:, cc, :, 1 + gh:9 + gh] \
                                                .transpose([0, 2, 1])
                                        else:
                                            rhs = x2[bi][:, cc, gh:gh + 8, 1:65]
                                        nc.tensor.matmul(
                                            ps_[:], wt_[:, cc, 128 * oc:128 * oc + 128],
                                            rhs, start=(cc == 0), stop=(cc == 1))
                                    if ti == 0:
                                        nc.vector.tensor_copy(
                                            sb_[:, oc, 8 * hf:8 * hf + 8, :], ps_[:])
                                    else:
                                        nc.scalar.copy(
                                            sb_[:, oc, 8 * hf:8 * hf + 8, :], ps_[:])
                            qk_sb.append(sb_)
                        q_sb, k_sb = qk_sb

                        # ---- vT for all 16 groups (M=64 matmuls at partition 0)
                        vt_sb = wp.tile([64, 16, 128], F32, tag="vt_sb", bufs=1)
                        for half in range(2):
                            vps = pp.tile([64, 8, 128], F32, tag="vt", bufs=1,
                                          name="ps_vt")
                            for sl in range(8):
                                gg = g0 + 8 * half + sl
                                if axis == "h":
                                    lhs_main = bpool_t[:, bi, :, gg]
                                else:
                                    lhs_main = b2_t[:, bi, gg, :]
                                nc.tensor.matmul(vps[:, sl, :], lhs_main,
                                                 wv_t[:], start=True, stop=True)
                            nc.vector.tensor_copy(vt_sb[:, 8 * half:8 * half + 8, :],
                                                  vps[:])

                        # ---- logits for 8 pairs, i packed on partition halves
                        lps = pp.tile([128, 8, 64], F32, tag="L", bufs=1, name="ps_L")
                        for pr in range(8):
                            for gi in range(2):
                                for oc in range(2):
                                    nc.tensor.matmul(
                                        lps[64 * gi:64 * gi + 64, pr, :],
                                        q_sb[:, oc, 2 * pr + gi, :].bitcast(F32),
                                        k_sb[:, oc, 2 * pr + gi, :].bitcast(F32),
                                        start=(oc == 0), stop=(oc == 1),
                                        tile_position=(0, 64 * gi))
                        # ---- batched softmax over free dim
                        negmx = wp.tile([128, 8], F32, tag="negmx")
                        nc.vector.tensor_reduce(negmx, lps[:], AX.X, ALU.max,
                                                negate=True)
                        pex = wp.tile([128, 8, 64], F32, tag="pex")
                        nc.vector.tensor_tensor(
                            pex[:], lps[:],
                            negmx[:, :, None].to_broadcast([128, 8, 64]), ALU.add)
                        nc.scalar.activation(pex[:], pex[:], AT.Exp, bias=0.0,
                                             scale=1.0)
                        sums = wp.tile([128, 8], F32, tag="sums")
                        nc.vector.tensor_reduce(sums, pex[:], AX.X, ALU.add)
                        rec = wp.tile([128, 8], F32, tag="rec")
                        nc.vector.reciprocal(rec, sums[:])
                        attn = pex
                        nc.vector.tensor_tensor(
                            attn[:], pex[:],
                            rec[:, :, None].to_broadcast([128, 8, 64]), ALU.mult)
                        row = bi * 64 + g0
                        nc.sync.dma_start(
                            attn_out[row:row + 16]
                            .rearrange("(pr g) i j -> (g i) pr j", g=2),
                            attn[:])
                        # ---- attn^T per pair: full [128,64] -> [64,128] PE
                        atps = pp.tile([64, 8, 128], F32, tag="misc", bufs=1,
                                       name="ps_at")
                        for pr in range(8):
                            nc.tensor.transpose(atps[:, pr, :], attn[:, pr, :],
                                                ident_t[:])
                        at_sb = wp.tile([64, 8, 128], F32, tag="at_sb", bufs=1)
                        nc.scalar.copy(at_sb[:], atps[:])
                        # ---- out = vT^T @ attnT (K=64 at partition 0)
                        ops_ = pp.tile([128, 16, 64], F32, tag="misc", bufs=1,
                                       name="ps_o")
                        for pr in range(8):
                            for gi in range(2):
                                nc.tensor.matmul(ops_[:, 2 * pr + gi, :],
                                                 vt_sb[:, 2 * pr + gi, :],
                                                 at_sb[:, pr, 64 * gi:64 * gi + 64],
                                                 start=True, stop=True)
                        if axis == "h":
                            nc.scalar.copy(
                                b2_t[:, bi, :, g0:g0 + 16].transpose([0, 2, 1]),
                                ops_[:])
                        else:
                            ost = wp.tile([128, 16, 64], F32R, tag="ost", bufs=1)
                            nc.scalar.copy(ost[:], ops_[:])
                            nc.sync.dma_start(out_dr[bi, :, g0:g0 + 16, :], ost[:])

        # ================= phase 3: height attention =================
        with nc.named_scope("attn_h"):
            attention_pass("h")
        if DEBUG_TAPS:
            for bi in range(BL):
                nc.sync.dma_start(dbg["b2"][bi], b2_t[:, bi])

        # fold width-pass v positional bias: b2 += pe2_w (bcast over h)
        for bi in range(BL):
            nc.vector.tensor_tensor(
                b2_t[:, bi], b2_t[:, bi],
                pe2w_t[:, None, :].to_broadcast([128, 64, 64]), ALU.add)

        # ================= phase 4: x3 = x2 + pe1_w; spill =================
        for bi in range(BL):
            nc.vector.tensor_tensor(
                x2[bi][:, :, :, 1:65], x2[bi].bitcast(F32)[:, :, :, 1:65],
                pe1w_t[:, :, None, :].to_broadcast([128, 2, 64, 64]), ALU.add)

        # ================= phase 5: width attention =================
        with nc.named_scope("attn_w"):
            attention_pass("w")

        # spill x3 to DRAM (whole rows; pad cols are zero in SBUF)
        for bi in range(BL):
            for cc in range(2):
                nc.sync.dma_start(x3_dr[bi, cc], x2[bi][:, cc])

        xpool.release()
        bppool.release()
        b2pool.release()

        # ================= phase 6+7: gating + fused conv pipeline =========
        NCH = 16            # chunks per image
        RS = 8              # rows per chunk
        with tc.tile_pool(name="cv", bufs=3) as cv, \
             tc.tile_pool(name="gpool", bufs=1) as gp, \
             tc.tile_pool(name="cps", bufs=1, space="PSUM") as cps:
            for bi in range(BL):
                sc = nc.enter_named_scope(f"conv_{bi}", False)
                # ---- gating: g = sigmoid(bn(Wa@out + ba)) -> g_pad [c,66,66]
                g_pad = gp.tile([128, 66, 66], F32, tag="g_pad")
                nc.gpsimd.memset(g_pad[:], 0.0)
                for q in range(8):
                    go = cv.tile([128, 8, 64], F32R, tag="go", bufs=1)
                    nc.sync.dma_start(go[:], out_dr[bi, :, 8 * q:8 * q + 8, :])
                    gps = cps.tile([128, 8, 64], F32, tag="g", bufs=2, name="ps_g")
                    nc.tensor.matmul(gps[:], waT_t[:], go[:], start=True, stop=True)
                    nc.scalar.activation(g_pad[:, 1 + 8 * q:9 + 8 * q, 1:65], gps[:],
                                         AT.Sigmoid, bias=vec_t["v_gb"][:],
                                         scale=vec_t["v_gs"][:])

                if DEBUG_TAPS:
                    nc.sync.dma_start(dbg["g"][bi], g_pad[:, 1:65, 1:65])

                # chunk tiles kept by index for boundary-row copies
                upc_tiles, catc_tiles, y1c_tiles = {}, {}, {}

                def produce_up_chunk(j):
                    """A(j): up rows 8j..8j+7 -> upc slots 1..8 (+slot0 copy)."""
                    xc = cv.tile([128, 2, 6, 66], F32R, tag="xc", name="xc", bufs=2)
                    r_lo = 4 * j - 1
                    for cc in range(2):
                        if j == 0:
                            nc.gpsimd.memset(xc.bitcast(F32)[:, cc, 0, :], 0.0)
                            nc.sync.dma_start(xc[:, cc, 1:6, :],
                                              x3_dr[bi, cc, :, 0:5, :])
                        elif j == 15:
                            nc.gpsimd.memset(xc.bitcast(F32)[:, cc, 5, :], 0.0)
                            nc.sync.dma_start(xc[:, cc, 0:5, :],
                                              x3_dr[bi, cc, :, 59:64, :])
                        else:
                            nc.sync.dma_start(xc[:, cc],
                                              x3_dr[bi, cc, :, r_lo:r_lo + 6, :])
                    m0 = 4 * j
                    tall = cv.tile([128, 2, 8, 66], F32, tag="tall", name="tall", bufs=2)
                    # h-lerp evens (rows 2m: src rows m-1, m -> slots 0..3 / 1..4)
                    wa_ = wtab_t[:, 0, m0:m0 + 4][:, None, :, None] \
                        .to_broadcast([128, 2, 4, 66])
                    wb_ = wtab_t[:, 1, m0:m0 + 4][:, None, :, None] \
                        .to_broadcast([128, 2, 4, 66])
                    t1 = cv.tile([128, 2, 4, 66], F32, tag="hl1", name="hl1", bufs=2)
                    nc.vector.tensor_tensor(t1[:], xc.bitcast(F32)[:, :, 0:4, :], wa_, ALU.mult)
                    nc.vector.tensor_tensor(tall[:, :, 0:8:2, :],
                                            xc.bitcast(F32)[:, :, 1:5, :], wb_, ALU.mult)
                    nc.vector.tensor_tensor(tall[:, :, 0:8:2, :],
                                            tall[:, :, 0:8:2, :], t1[:], ALU.add)
                    # h-lerp odds (rows 2m+1: src rows m, m+1 -> slots 1..4 / 2..5)
                    wc_ = wtab_t[:, 2, m0:m0 + 4][:, None, :, None] \
                        .to_broadcast([128, 2, 4, 66])
                    wd_ = wtab_t[:, 3, m0:m0 + 4][:, None, :, None] \
                        .to_broadcast([128, 2, 4, 66])
                    t3 = cv.tile([128, 2, 4, 66], F32, tag="hl3", name="hl3", bufs=2)
                    nc.gpsimd.tensor_tensor(t3[:], xc.bitcast(F32)[:, :, 1:5, :], wc_, ALU.mult)
                    nc.gpsimd.tensor_tensor(tall[:, :, 1:8:2, :],
                                            xc.bitcast(F32)[:, :, 2:6, :], wd_, ALU.mult)
                    nc.gpsimd.tensor_tensor(tall[:, :, 1:8:2, :],
                                            tall[:, :, 1:8:2, :], t3[:], ALU.add)

                    upc = cv.tile([128, 2, 10, 130], F32R, tag="upc", name="upc")
                    # w-lerp evens -> cols 1,3,..,127; odds -> cols 2,4,..,128
                    we0 = wtab_t[:, 0, :][:, None, None, :].to_broadcast([128, 2, 8, 64])
                    we1 = wtab_t[:, 1, :][:, None, None, :].to_broadcast([128, 2, 8, 64])
                    wo0 = wtab_t[:, 2, :][:, None, None, :].to_broadcast([128, 2, 8, 64])
                    wo1 = wtab_t[:, 3, :][:, None, None, :].to_broadcast([128, 2, 8, 64])
                    u1 = cv.tile([128, 2, 8, 64], F32, tag="wl1", name="wl1", bufs=2)
                    ue = upc[:, :, 1:9, 1:129:2]
                    nc.vector.tensor_tensor(u1[:], tall[:, :, :, 0:64], we0, ALU.mult)
                    nc.vector.tensor_tensor(ue, tall[:, :, :, 1:65], we1, ALU.mult)
                    nc.vector.tensor_tensor(ue, ue.bitcast(F32), u1[:], ALU.add)
                    u3 = cv.tile([128, 2, 8, 64], F32, tag="wl3", name="wl3", bufs=2)
                    u4 = cv.tile([128, 2, 8, 64], F32, tag="wl4", name="wl4", bufs=2)
                    uo = upc[:, :, 1:9, 2:130:2]
                    nc.gpsimd.tensor_tensor(u3[:], tall[:, :, :, 1:65], wo0, ALU.mult)
                    nc.vector.tensor_tensor(u4[:], tall[:, :, :, 2:66], wo1, ALU.mult)
                    nc.vector.tensor_tensor(uo, u3[:], u4[:], ALU.add)
                    # pad cols 0 and 129 of fresh slot rows
                    for cc in range(2):
                        nc.gpsimd.memset(upc.bitcast(F32)[:, cc, :, 0], 0.0)
                        nc.gpsimd.memset(upc.bitcast(F32)[:, cc, :, 129], 0.0)
                    # slot 0 <- previous chunk's slot 8 (row 8j-1)
                    if j == 0:
                        for cc in range(2):
                            nc.gpsimd.memset(upc.bitcast(F32)[:, cc, 0, 1:129], 0.0)
                    else:
                        nc.scalar.copy(upc[:, :, 0:1, 1:129],
                                       upc_tiles[j - 1][:, :, 8:9, 1:129])
                    upc_tiles[j] = upc
                    return upc

                def conv_boundary(jj, src_tiles, ncc):
                    """Fill slot 9 (row 8jj+8) of window jj from next chunk."""
                    win = src_tiles[jj]
                    if jj == 15:
                        if ncc == 2:
                            for cc in range(2):
                                nc.gpsimd.memset(win.bitcast(F32)[:, cc, 9, 1:129], 0.0)
                        else:
                            nc.gpsimd.memset(win.bitcast(F32)[:, 9, 1:129], 0.0)
                    else:
                        nxt = src_tiles[jj + 1]
                        if ncc == 2:
                            nc.scalar.copy(win[:, :, 9:10, 1:129],
                                           nxt[:, :, 1:2, 1:129])
                        else:
                            nc.scalar.copy(win[:, 9:10, 1:129],
                                           nxt[:, 1:2, 1:129])

                def conv_batch(jj, q, src_tiles, w_t, ncc, ptag, epifn):
                    win = src_tiles[jj]
                    ps = cps.tile([128, 4, 128], F32, tag=ptag, bufs=2,
                                  name=f"ps_{ptag}")
                    first = True
                    for dy in range(3):
                        for dx in range(3):
                            for cc in range(ncc):
                                if ncc == 2:
                                    rhs = win[:, cc, 4 * q + dy:4 * q + dy + 4,
                                              dx:dx + 128]
                                    lhs = w_t[:, dy, dx, cc, :]
                                else:
                                    rhs = win[:, 4 * q + dy:4 * q + dy + 4,
                                              dx:dx + 128]
                                    lhs = w_t[:, dy, dx, :]
                                last = (dy == 2 and dx == 2 and cc == ncc - 1)
                                nc.tensor.matmul(ps[:], lhs, rhs,
                                                 start=first, stop=last)
                                first = False
                    epifn(ps, q)

                def alloc_cat_chunk(jj):
                    catc = cv.tile([128, 2, 10, 130], F32R, tag="catc", name="catc")
                    for cc in range(2):
                        nc.gpsimd.memset(catc.bitcast(F32)[:, cc, :, 0], 0.0)
                        nc.gpsimd.memset(catc.bitcast(F32)[:, cc, :, 129], 0.0)
                    if jj == 0:
                        for cc in range(2):
                            nc.gpsimd.memset(catc.bitcast(F32)[:, cc, 0, 1:129], 0.0)
                    else:
                        nc.scalar.copy(catc[:, :, 0:1, 1:129],
                                       catc_tiles[jj - 1][:, :, 8:9, 1:129])
                    catc_tiles[jj] = catc

                def convup_batch(jj, q):
                    catc = catc_tiles[jj]

                    def epi_up(ps, q_):
                        nc.scalar.activation(
                            catc[:, 0, 1 + 4 * q_:5 + 4 * q_, 1:129], ps[:],
                            AT.Relu, bias=vec_t["v_bup"][:], scale=1.0)
                    conv_batch(jj, q, upc_tiles, wup_t, 2, "up", epi_up)

                def make_bout_chunk(jj):
                    catc = catc_tiles[jj]
                    # ---- b_out rows 8jj..8jj+7
                    m0 = 4 * jj
                    tg = cv.tile([128, 8, 66], F32, tag="tg", name="tg", bufs=1)
                    wa_ = wtab_t[:, 0, m0:m0 + 4][:, :, None].to_broadcast([128, 4, 66])
                    wb_ = wtab_t[:, 1, m0:m0 + 4][:, :, None].to_broadcast([128, 4, 66])
                    wc_ = wtab_t[:, 2, m0:m0 + 4][:, :, None].to_broadcast([128, 4, 66])
                    wd_ = wtab_t[:, 3, m0:m0 + 4][:, :, None].to_broadcast([128, 4, 66])
                    g1 = cv.tile([128, 4, 66], F32, tag="gl1", name="gl1", bufs=2)
                    nc.vector.tensor_tensor(g1[:], g_pad[:, m0:m0 + 4, :], wa_, ALU.mult)
                    nc.vector.tensor_tensor(tg[:, 0:8:2, :],
                                            g_pad[:, m0 + 1:m0 + 5, :], wb_, ALU.mult)
                    nc.vector.tensor_tensor(tg[:, 0:8:2, :], tg[:, 0:8:2, :], g1[:], ALU.add)
                    g3 = cv.tile([128, 4, 66], F32, tag="gl3", name="gl3", bufs=2)
                    nc.gpsimd.tensor_tensor(g3[:], g_pad[:, m0 + 1:m0 + 5, :], wc_, ALU.mult)
                    nc.gpsimd.tensor_tensor(tg[:, 1:8:2, :],
                                            g_pad[:, m0 + 2:m0 + 6, :], wd_, ALU.mult)
                    nc.gpsimd.tensor_tensor(tg[:, 1:8:2, :], tg[:, 1:8:2, :], g3[:], ALU.add)
                    upg = cv.tile([128, 8, 130], F32, tag="upg", name="upg", bufs=1)
                    we0 = wtab_t[:, 0, :][:, None, :].to_broadcast([128, 8, 64])
                    we1 = wtab_t[:, 1, :][:, None, :].to_broadcast([128, 8, 64])
                    wo0 = wtab_t[:, 2, :][:, None, :].to_broadcast([128, 8, 64])
                    wo1 = wtab_t[:, 3, :][:, None, :].to_broadcast([128, 8, 64])
                    q1 = cv.tile([128, 8, 64], F32, tag="ql1", name="ql1", bufs=2)
                    ge = upg[:, :, 1:129:2]
                    nc.vector.tensor_tensor(q1[:], tg[:, :, 0:64], we0, ALU.mult)
                    nc.vector.tensor_tensor(ge, tg[:, :, 1:65], we1, ALU.mult)
                    nc.vector.tensor_tensor(ge, ge, q1[:], ALU.add)
                    q3 = cv.tile([128, 8, 64], F32, tag="ql3", name="ql3", bufs=2)
                    go_ = upg[:, :, 2:130:2]
                    nc.gpsimd.tensor_tensor(q3[:], tg[:, :, 1:65], wo0, ALU.mult)
                    nc.gpsimd.tensor_tensor(go_, tg[:, :, 2:66], wo1, ALU.mult)
                    nc.gpsimd.tensor_tensor(go_, go_, q3[:], ALU.add)
                    bt = cv.tile([128, 8, 128], F32, tag="bt", name="bt", bufs=2)
                    nc.sync.dma_start(bt[:], b_dr[bi, :, 8 * jj:8 * jj + 8, :])
                    nc.vector.tensor_tensor(catc[:, 1, 1:9, 1:129],
                                            upg[:, :, 1:129], bt[:], ALU.mult)
                    if DEBUG_TAPS:
                        bo_sb = cv.tile([128, 8, 128], F32, tag="dbgbo")
                        nc.vector.tensor_copy(bo_sb[:], catc.bitcast(F32)[:, 1, 1:9, 1:129])
                        nc.sync.dma_start(dbg["bout"][bi, :, 8 * jj:8 * jj + 8, :], bo_sb[:])

                def alloc_y1_chunk(jj):
                    y1c = cv.tile([128, 10, 130], F32R, tag="y1c", name="y1c", bufs=2)
                    nc.gpsimd.memset(y1c.bitcast(F32)[:, :, 0], 0.0)
                    nc.gpsimd.memset(y1c.bitcast(F32)[:, :, 129], 0.0)
                    if jj == 0:
                        nc.gpsimd.memset(y1c.bitcast(F32)[:, 0:1, 1:129], 0.0)
                    else:
                        nc.scalar.copy(y1c[:, 0:1, 1:129],
                                       y1c_tiles[jj - 1][:, 8:9, 1:129])
                    y1c_tiles[jj] = y1c

                def c1_batch(jj, q):
                    y1c = y1c_tiles[jj]

                    def epi_c1(ps, q_):
                        nc.scalar.activation(y1c[:, 1 + 4 * q_:5 + 4 * q_, 1:129],
                                             ps[:], AT.Relu,
                                             bias=vec_t["v_c1b"][:],
                                             scale=vec_t["v_c1s"][:])
                    conv_batch(jj, q, catc_tiles, wc1_t, 2, "c1", epi_c1)

                yst_tiles = {}

                def c2_batch(jj, q):
                    if q == 0:
                        yst_tiles[jj] = cv.tile([128, 8, 128], F32, tag="yst",
                                                name="yst", bufs=2)
                    yst = yst_tiles[jj]

                    def epi_c2(ps, q_):
                        nc.scalar.activation(yst[:, 4 * q_:4 * q_ + 4, :], ps[:],
                                             AT.Relu, bias=vec_t["v_c2b"][:],
                                             scale=vec_t["v_c2s"][:])
                    conv_batch(jj, q, y1c_tiles, wc2_t, 1, "c2", epi_c2)
                    if q == 1:
                        nc.sync.dma_start(y_dr[bi, :, 8 * jj:8 * jj + 8, :], yst[:])

                # ---- 4-deep chunk-skewed pipeline; the A(j)-dependent
                # conv-up batch q1 is emitted LAST so the in-order PE queue
                # is not blocked by the lerp producers.
                for j in range(NCH + 3):
                    if j < NCH:
                        produce_up_chunk(j)
                    if 1 <= j <= NCH:
                        alloc_cat_chunk(j - 1)
                        conv_boundary(j - 1, upc_tiles, 2)
                        convup_batch(j - 1, 0)
                        make_bout_chunk(j - 1)
                    if 2 <= j <= NCH + 1:
                        alloc_y1_chunk(j - 2)
                        conv_boundary(j - 2, catc_tiles, 2)
                        c1_batch(j - 2, 0)
                    if 3 <= j <= NCH + 2:
                        conv_boundary(j - 3, y1c_tiles, 1)
                        c2_batch(j - 3, 0)
                        c2_batch(j - 3, 1)
                    if 2 <= j <= NCH + 1:
                        c1_batch(j - 2, 1)
                    if 1 <= j <= NCH:
                        convup_batch(j - 1, 1)
                nc.leave_named_scope(f"conv_{bi}", sc[0], False)
        cpool.release()
    nc.compile()
    return nc


# ------------------------------------------------------------------ runtime
_CACHE = {}


def _get_program():
    if "nc" not in _CACHE:
        _CACHE["nc"] = build_program()
    return _CACHE["nc"]


def kernel(**inputs):
    inputs = {k: np.asarray(v, dtype=np.float32) for k, v in inputs.items()}
    nc = _get_program()
    consts = _host_consts(inputs)
    in_maps = []
    for c in range(NCORES):
        m = dict(consts)
        m["x"] = inputs["x"][c * BL:(c + 1) * BL]
        m["b"] = inputs["b"][c * BL:(c + 1) * BL]
        in_maps.append(m)
    res = run_bass_kernel_spmd(nc, in_maps, core_ids=list(range(NCORES)),
                               trace=False)
    y = np.concatenate([res.results[c]["y"] for c in range(NCORES)], axis=0)
    ah = np.concatenate([res.results[c]["attn_h"] for c in range(NCORES)], axis=0)
    aw = np.concatenate([res.results[c]["attn_w"] for c in range(NCORES)], axis=0)
    _CACHE["last_results"] = res
    return y, ah, aw
